# revision 1
# baseline (speedup 1.0000x reference)
"""GAT (2-layer, heads=1) on 8 Trainium2 NeuronCores.

Strategy (1D node partition):
  - Nodes are split into 8 chunks of NL; core c owns dst-chunk c.
  - Layer tables h^T/alpha_src are computed per-chunk (feature-major via
    PE matmuls on x^T), AllGathered, and kept in SBUF as a per-partition-group
    table: partition group g (16 partitions) holds (h|alpha_src) of chunk g,
    packed two fp16 per fp32 word.
  - Edges are bucketed on the host by (dst-core, src-chunk, dst-block) and
    sorted by dst. Each edge tile covers one block of NB dst nodes for all
    8 src-chunk groups at once; per-edge gathers run on GPSIMD (ap_gather),
    per-edge math on DVE/ACT, and segment-sums via masked tensor_tensor_scan
    (state = mask*state + w, mask=0 at segment starts) + a boundary gather at
    each node's last edge. Group partials are combined with a matmul against
    a 0/1 selection matrix; softmax normalization divides at the node level.
  - The segment-start mask is derived ON DEVICE: the alpha_dst table packs
    (dst-local-id | alpha_dst) as two fp16 lanes per word, so one gather
    yields both, and mask = is_equal(dstid[e], dstid[e-1]).
  - Slot 0 of every bucket is a sentinel; nodes with no edges in a bucket
    point their boundary index at slot 0 (whose running sum is always 0).
  - Softmax max-subtraction is skipped: attention logits here are O(1), and
    alpha = exp(e)/sum(exp(e)) is shift-invariant.
  - All bulk HBM traffic goes through SWDGE (nc.gpsimd.dma_start) so it
    sprays across all 16 SDMA engines instead of serializing on the single
    HWDGE dynamic ring.

Host preprocessing only reorders/buckets edge indices (structure), never
touches float data dependent on device results.

Performance state (measured via NTFF traces): runtime is >90% the three
per-tile GPSIMD ap_gathers (~28ns/index serial on the Pool engine, Q7
RD_CMD latency-bound): h|as gather (T_e idxs) + ad|dstid gather (T_e) +
boundary gather (512) ~= 136us/tile. Paths explored to break this floor:
  - indirect_dma_start per-edge compact gather: runs fast (4.9-5.2ms e2e)
    but HW consumes ONE OFFSET PER CONTIGUOUS DEST RUN (not per element,
    unlike bass_interp); strided dests are rejected ("DMA APs must be
    contiguous in last dimension"), so per-element indirect gathers are
    unreachable with this API. Multi-writer DRAM deps also exceed the
    indirect instruction's wait slots unless staged behind a single
    consolidating DMA (see kernel_v5.py).
  - Remaining candidates: dma_gather (CounterMachine SWDGE, 16-partition
    wrapped idx convention) with a node-major DRAM table + DRAM-roundtrip
    relayout, or a full edge-major redesign with degree-binned dst
    striping (regular segment boundaries, no boundary gather).
"""

import math
import numpy as np

from concourse import bass, bacc, mybir
import concourse.tile as tile

F32 = mybir.dt.float32
BF16 = mybir.dt.bfloat16
F16 = mybir.dt.float16
I16 = mybir.dt.int16
I32 = mybir.dt.int32

NEG_SLOPE = 0.2
# fp32 word 0xF7500000: low fp16 lane (h) = 0, high fp16 lane
# (alpha_src) = -29952 -> exp(lrelu(...)) == 0, so sentinel edges vanish.
SENT_PACKED = float(
    np.frombuffer(
        (np.uint32(np.float16(-29952.0).view(np.uint16)) << np.uint32(16)).tobytes(),
        np.float32,
    )[0]
)

FULL_CFG = dict(
    NCORES=8, N=100000, F=512, H=16,
    NL=12500, NB=500, NT=25, CH=500, NCH=25,
)


# ---------------------------------------------------------------- host prep

def _round_up(x, m):
    return (x + m - 1) // m * m


def host_prep(edge_index, cfg):
    """Bucket and sort edges; build device index streams (no float data).

    Returns (T_e, per_core) where per_core[c] has 'idxs' [128, NT*IW] i16.
    """
    NC, N, NL = cfg["NCORES"], cfg["N"], cfg["NL"]
    NB, NT = cfg["NB"], cfg["NT"]
    G = 8

    # Self-loops are NOT added here: their contribution is computed
    # analytically at the node level on-device (no gather needed).
    src = np.asarray(edge_index[0], dtype=np.int64)
    dst = np.asarray(edge_index[1], dtype=np.int64)

    core = dst // NL
    grp = src // NL
    order = np.lexsort((src, dst, grp, core))
    src, dst, core, grp = (a[order] for a in (src, dst, core, grp))
    blk = (dst % NL) // NB

    bucket = (core * G + grp) * NT + blk
    nbuck = NC * G * NT
    counts = np.bincount(bucket, minlength=nbuck)
    # +1: slot 0 of every bucket is a sentinel (known-zero running sum)
    T_e = _round_up(int(counts.max()) + 17, 128)
    assert T_e <= 32767

    starts = np.zeros(nbuck, dtype=np.int64)
    starts[1:] = np.cumsum(counts)[:-1]
    pos = np.arange(src.size) - starts[bucket] + 1

    is_last = np.ones(src.size, dtype=bool)
    is_last[:-1] = ~((bucket[1:] == bucket[:-1]) & (dst[1:] == dst[:-1]))

    srcl = np.full((NC, G, NT, T_e), NL, dtype=np.int16)
    dstl = np.full((NC, G, NT, T_e), NB, dtype=np.int16)
    bnd = np.zeros((NC, G, NT, 512), dtype=np.int16)  # default: sentinel slot 0

    c_, g_, b_ = core, grp, blk
    srcl[c_, g_, b_, pos] = (src % NL).astype(np.int16)
    dstl[c_, g_, b_, pos] = ((dst % NL) % NB).astype(np.int16)
    bnd[c_[is_last], g_[is_last], b_[is_last],
        ((dst % NL) % NB)[is_last]] = pos[is_last].astype(np.int16)

    def wrap(a, w):
        # [NC, G, NT, w] -> [NC, 128, NT, w//16]; w16[c, 16g+p, t, s] = a[c, g, t, s*16+p]
        n = a.shape[-1]
        return (a.reshape(NC, G, NT, n // 16, 16)
                 .transpose(0, 1, 4, 2, 3)
                 .reshape(NC, 128, NT, n // 16))

    src_w, dst_w, bnd_w = wrap(srcl, T_e), wrap(dstl, T_e), wrap(bnd, 512)
    idxs = np.concatenate([src_w, dst_w, bnd_w], axis=3)  # [NC,128,NT,IW]
    IW = idxs.shape[3]
    idxs = np.ascontiguousarray(idxs.reshape(NC, 128, NT * IW))

    per_core = [{"idxs": idxs[c]} for c in range(NC)]
    return T_e, per_core


# ------------------------------------------------------------- device build

def build_nc(cfg, T_e, max_waits=2, ctrl_max_waits=1, split=True):
    NC, N, F, H = cfg["NCORES"], cfg["N"], cfg["F"], cfg["H"]
    NL, NB, NT = cfg["NL"], cfg["NB"], cfg["NT"]
    CH, NCH = cfg["CH"], cfg["NCH"]
    KB = F // 128
    NLP = NL + 16               # table width incl. sentinel
    NBW = NB + 16               # per-tile alpha_dst table width
    T16 = T_e // 16
    IW = 2 * T16 + 512 // 16
    NBLK = math.ceil(NL / 128)
    NL2 = NBLK * 128
    W2 = NL2 // 2               # fp32 words backing the bf16 out1T row
    rg = [list(range(NC))]

    nc = bacc.Bacc("TRN2", target_bir_lowering=False)

    xtw = nc.declare_dram_parameter("xtw", [128, NCH * KB * CH], F32, isOutput=False)
    w1 = nc.declare_dram_parameter("w1", [F, H], F32, isOutput=False)
    w2 = nc.declare_dram_parameter("w2", [H, H], F32, isOutput=False)
    a1rep = nc.declare_dram_parameter("a1rep", [H, 16], F32, isOutput=False)
    ad1rep = nc.declare_dram_parameter("ad1rep", [H, 16], F32, isOutput=False)
    a2rep = nc.declare_dram_parameter("a2rep", [H, 16], F32, isOutput=False)
    ad2rep = nc.declare_dram_parameter("ad2rep", [H, 16], F32, isOutput=False)
    b1p = nc.declare_dram_parameter("b1p", [H, 1], F32, isOutput=False)
    b2p = nc.declare_dram_parameter("b2p", [H, 1], F32, isOutput=False)
    selp = nc.declare_dram_parameter("selp", [128, 16], F32, isOutput=False)
    identp = nc.declare_dram_parameter("identp", [16, 16], F32, isOutput=False)
    dstidp = nc.declare_dram_parameter("dstidp", [1, NBW], F32, isOutput=False)
    idxsp = nc.declare_dram_parameter("idxs", [128, NT * IW], I16, isOutput=False)
    outp = nc.declare_dram_parameter("out", [128, NBLK * H], F32, isOutput=True)

    ag_in = [nc.dram_tensor(f"ag_in{l}", [16, NL], F32) for l in (1, 2)]
    ag_out = [nc.dram_tensor(f"ag_out{l}", [128, NL], F32, addr_space="Shared")
              for l in (1, 2)]
    # per-node fp32 words packing (alpha_dst | dst-local-id) as fp16 lanes
    ad_row = [nc.dram_tensor(f"ad_row{l}", [1, NLP], F32) for l in (1, 2)]

    with tile.TileContext(nc, num_cores=NC) as tc:
        with tc.tile_pool(name="const", bufs=1) as cpool:
            w1t = cpool.tile([128, KB, H], BF16)
            nc.gpsimd.dma_start(out=w1t[:], in_=w1[:].rearrange("(b p) h -> p b h", p=128))
            # w2 / identity copies aligned to the partitions where out1T /
            # out2T live (matmul needs lhsT and rhs on the same partitions).
            w2t = cpool.tile([48, H], BF16)
            nc.gpsimd.dma_start(out=w2t[32:48, :], in_=w2[:])
            a1t = cpool.tile([16, 16], BF16)
            nc.gpsimd.dma_start(out=a1t[:], in_=a1rep[:])
            ad1t = cpool.tile([16, 16], BF16)
            nc.gpsimd.dma_start(out=ad1t[:], in_=ad1rep[:])
            a2t = cpool.tile([16, 16], BF16)
            nc.gpsimd.dma_start(out=a2t[:], in_=a2rep[:])
            ad2t = cpool.tile([16, 16], BF16)
            nc.gpsimd.dma_start(out=ad2t[:], in_=ad2rep[:])
            b1t = cpool.tile([16, 1], F32)
            nc.sync.dma_start(out=b1t[:], in_=b1p[:])
            b2t = cpool.tile([16, 1], F32)
            nc.sync.dma_start(out=b2t[:], in_=b2p[:])
            selt = cpool.tile([128, 16], F16)
            nc.gpsimd.dma_start(out=selt[:], in_=selp[:])
            idt = cpool.tile([80, 16], F32)
            nc.sync.dma_start(out=idt[64:80, :], in_=identp[:])

            # Stacked per-layer state, one tile so 16-partition rows share
            # column space; engine accesses must start at partition 0/32/64/96:
            # p0-15 wself, p32-47 out1T (bf16, matmul rhs -> base 32),
            # p64-79 out2T (transpose input -> base 64), p96-111 pself.
            stk = cpool.tile([128, NL2], F32)
            wselfv = stk[0:16, 0:NL]
            pselfv = stk[96:112, 0:NL]
            out1v = stk[32:48, 0:W2].bitcast(BF16)      # [16, NL2] bf16
            out2v = stk[64:80, 0:NL2]
            nc.vector.memset(stk[64:80, NL:], 0.0)

            # iota 0..NB-1 (dst-local ids, fp16) for the packed ad_row build
            iot = cpool.tile([1, NB], F16)
            nc.gpsimd.dma_start(out=iot[:], in_=dstidp[0:1, 0:NB])
            zero16 = cpool.tile([1, 16], F32)
            nc.vector.memset(zero16[:], 0.0)

            idxs_sb = cpool.tile([128, NT * IW], I16)
            nc.gpsimd.dma_start(out=idxs_sb[:], in_=idxsp[:])

            # ---------------- phase A (layer 1): tables from x^T ----------
            with (
                tc.tile_pool(name="pa", bufs=2) as pa,
                tc.tile_pool(name="pap", bufs=2, space="PSUM") as pap,
            ):
                for c in range(NCH):
                    sl = slice(c * CH, (c + 1) * CH)
                    xt_t = pa.tile([128, KB, CH], BF16, tag="xt")
                    nc.gpsimd.dma_start(
                        out=xt_t[:],
                        in_=xtw[:, c * KB * CH:(c + 1) * KB * CH]
                        .rearrange("p (b n) -> p b n", b=KB))
                    ph = pap.tile([16, CH], F32, tag="ph")
                    for b in range(KB):
                        nc.tensor.matmul(ph[:], lhsT=w1t[:, b, :], rhs=xt_t[:, b, :],
                                         start=(b == 0), stop=(b == KB - 1))
                    hch = pa.tile([16, CH], BF16, tag="hch")
                    nc.scalar.copy(hch[:], ph[:])
                    pas = pap.tile([16, CH], F32, tag="pas")
                    nc.tensor.matmul(pas[:], lhsT=a1t[:], rhs=hch[:], start=True, stop=True)
                    pad_ = pap.tile([16, CH], F32, tag="pad")
                    nc.tensor.matmul(pad_[:], lhsT=ad1t[:], rhs=hch[:], start=True, stop=True)
                    packed = pa.tile([16, CH], F32, tag="packed")
                    pk = packed[:].bitcast(F16)
                    nc.vector.tensor_copy(pk[:, 0::2], hch[:])
                    nc.vector.tensor_copy(pk[:, 1::2], pas[:])
                    nc.gpsimd.dma_start(out=ag_in[0][:, sl], in_=packed[:])
                    adrow = pa.tile([1, CH], F32, tag="adrow")
                    ar16 = adrow[:].bitcast(F16)
                    nc.vector.tensor_copy(ar16[:, 0::2], pad_[0:1, :])
                    nc.vector.tensor_copy(ar16[:, 1::2], iot[:])
                    nc.gpsimd.dma_start(out=ad_row[0][0:1, sl], in_=adrow[:])
                    # analytic self-loop contribution for this chunk
                    adfull = pa.tile([16, CH], F32, tag="adfull")
                    nc.scalar.copy(adfull[:], pad_[:])
                    tself = pa.tile([16, CH], F32, tag="tself")
                    nc.vector.tensor_add(tself[:], pas[:], adfull[:])
                    nc.vector.scalar_tensor_tensor(
                        tself[:], tself[:], NEG_SLOPE, tself[:],
                        op0=mybir.AluOpType.mult, op1=mybir.AluOpType.max)
                    pselfc = pa.tile([16, CH], F32, tag="pselfc")
                    nc.scalar.activation(pselfc[:], tself[:],
                                         mybir.ActivationFunctionType.Exp)
                    nc.scalar.copy(pselfv[:, sl], pselfc[:])
                    nc.vector.tensor_mul(wselfv[:, sl], pselfc[:], hch[:])

            def run_layer(l, writer):
                nc.gpsimd.dma_start(out=ad_row[l][0:1, NL:], in_=zero16[:])
                nc.gpsimd.collective_compute(
                    "AllGather", mybir.AluOpType.bypass, replica_groups=rg,
                    ins=[ag_in[l][:]], outs=[ag_out[l][:]])
                with tc.tile_pool(name=f"tab{l}", bufs=1) as tabp:
                    table = tabp.tile([128, NLP], F32, tag="table")
                    nc.gpsimd.dma_start(out=table[:, :NL], in_=ag_out[l][:])
                    nc.vector.memset(table[:, NL:], SENT_PACKED)
                    with (
                        tc.tile_pool(name=f"ed{l}", bufs=2) as ed,
                        tc.tile_pool(name=f"eb{l}", bufs=3) as eb,
                        tc.tile_pool(name=f"e1{l}", bufs=1) as e1,
                        tc.tile_pool(name=f"edp{l}", bufs=2, space="PSUM") as edp,
                    ):
                        for t in range(NT):
                            o = t * IW
                            i_src = idxs_sb[:, o:o + T16]
                            i_dst = idxs_sb[:, o + T16:o + 2 * T16]
                            i_bnd = idxs_sb[:, o + 2 * T16:o + IW]
                            adbt = e1.tile([128, NBW], F32, tag="adbt", bufs=2)
                            nc.gpsimd.dma_start(
                                out=adbt[:],
                                in_=ad_row[l][0:1, t * NB:t * NB + NBW]
                                .to_broadcast([128, NBW]))
                            ghs = ed.tile([128, T_e], F32, tag="ghs")
                            nc.gpsimd.ap_gather(
                                ghs[:], table[:], i_src,
                                channels=128, num_elems=NLP, d=1, num_idxs=T_e)
                            adg = e1.tile([128, T_e], F32, tag="adg")
                            nc.gpsimd.ap_gather(
                                adg[:], adbt[:], i_dst,
                                channels=128, num_elems=NBW, d=1, num_idxs=T_e)
                            g16 = ghs[:].bitcast(F16)   # [128, 2*T_e]
                            a16 = adg[:].bitcast(F16)   # (ad | dstid) lanes
                            mask = e1.tile([128, T_e], F16, tag="mask")
                            nc.vector.memset(mask[:, 0:1], 0.0)
                            nc.vector.tensor_tensor(
                                mask[:, 1:], a16[:, 1::2][:, 1:],
                                a16[:, 1::2][:, :-1], op=mybir.AluOpType.is_equal)
                            tt = ed.tile([128, T_e], F16, tag="tt")
                            nc.vector.tensor_add(tt[:], g16[:, 1::2], a16[:, 0::2])
                            nc.vector.scalar_tensor_tensor(
                                tt[:], tt[:], NEG_SLOPE, tt[:],
                                op0=mybir.AluOpType.mult, op1=mybir.AluOpType.max)
                            pp = e1.tile([128, T_e], F16, tag="pp")
                            nc.scalar.activation(pp[:], tt[:],
                                                 mybir.ActivationFunctionType.Exp)
                            ww = ed.tile([128, T_e], F16, tag="ww")
                            nc.vector.tensor_mul(ww[:], pp[:], g16[:, 0::2])
                            sc = eb.tile([128, T_e], F32, tag="sc", bufs=1)
                            sc16 = sc[:].bitcast(F16)   # (w | p) lanes
                            nc.vector.tensor_tensor_scan(
                                sc16[:, 0::2], mask[:], ww[:], 0.0,
                                mybir.AluOpType.mult, mybir.AluOpType.add)
                            nc.vector.tensor_tensor_scan(
                                sc16[:, 1::2], mask[:], pp[:], 0.0,
                                mybir.AluOpType.mult, mybir.AluOpType.add)
                            bg = e1.tile([128, 512], F32, tag="bg")
                            nc.gpsimd.ap_gather(
                                bg[:], sc[:], i_bnd,
                                channels=128, num_elems=T_e, d=1, num_idxs=512)
                            bg16 = bg[:].bitcast(F16)
                            pu = edp.tile([16, 512], F32, tag="pu")
                            nc.tensor.matmul(pu[:], lhsT=selt[:], rhs=bg16[:, 0::2],
                                             start=True, stop=True)
                            ps = edp.tile([16, 512], F32, tag="ps")
                            nc.tensor.matmul(ps[:], lhsT=selt[:], rhs=bg16[:, 1::2],
                                             start=True, stop=True)
                            # add analytic self-loop terms, then normalize
                            den = ed.tile([16, NB], F32, tag="s16", bufs=4)
                            nc.vector.tensor_add(den[:], ps[:, :NB],
                                                 pselfv[:, t * NB:(t + 1) * NB])
                            rc = ed.tile([16, NB], F32, tag="s16", bufs=4)
                            nc.vector.reciprocal_approx_fast(out=rc[:], in_=den[:])
                            num = ed.tile([16, NB], F32, tag="s16", bufs=4)
                            nc.vector.tensor_add(num[:], pu[:, :NB],
                                                 wselfv[:, t * NB:(t + 1) * NB])
                            uv = ed.tile([16, NB], F32, tag="s16", bufs=4)
                            nc.vector.tensor_mul(uv[:], num[:], rc[:])
                            writer(t, uv)

            def write1(t, uv):
                nc.scalar.activation(out1v[:, t * NB:(t + 1) * NB], uv[:],
                                     mybir.ActivationFunctionType.Relu,
                                     bias=b1t[:, 0:1])

            def write2(t, uv):
                nc.vector.tensor_scalar_add(out2v[:, t * NB:(t + 1) * NB], uv[:],
                                            b2t[:, 0:1])

            run_layer(0, write1)

            # ---------------- phase A (layer 2): tables from out1T --------
            with (
                tc.tile_pool(name="pa2", bufs=2) as pa2,
                tc.tile_pool(name="pap2", bufs=2, space="PSUM") as pap2,
            ):
                for c in range(NCH):
                    sl = slice(c * CH, (c + 1) * CH)
                    ph = pap2.tile([16, CH], F32, tag="ph2")
                    nc.tensor.matmul(ph[:], lhsT=w2t[32:48, :], rhs=out1v[:, sl],
                                     start=True, stop=True)
                    h2ch = pa2.tile([16, CH], BF16, tag="h2ch")
                    nc.scalar.copy(h2ch[:], ph[:])
                    pas = pap2.tile([16, CH], F32, tag="pas2")
                    nc.tensor.matmul(pas[:], lhsT=a2t[:], rhs=h2ch[:], start=True, stop=True)
                    pad_ = pap2.tile([16, CH], F32, tag="pad2")
                    nc.tensor.matmul(pad_[:], lhsT=ad2t[:], rhs=h2ch[:], start=True, stop=True)
                    packed = pa2.tile([16, CH], F32, tag="packed2")
                    pk = packed[:].bitcast(F16)
                    nc.vector.tensor_copy(pk[:, 0::2], h2ch[:])
                    nc.vector.tensor_copy(pk[:, 1::2], pas[:])
                    nc.gpsimd.dma_start(out=ag_in[1][:, sl], in_=packed[:])
                    adrow = pa2.tile([1, CH], F32, tag="adrow2")
                    ar16 = adrow[:].bitcast(F16)
                    nc.vector.tensor_copy(ar16[:, 0::2], pad_[0:1, :])
                    nc.vector.tensor_copy(ar16[:, 1::2], iot[:])
                    nc.gpsimd.dma_start(out=ad_row[1][0:1, sl], in_=adrow[:])
                    adfull = pa2.tile([16, CH], F32, tag="adfull2")
                    nc.scalar.copy(adfull[:], pad_[:])
                    tself = pa2.tile([16, CH], F32, tag="tself2")
                    nc.vector.tensor_add(tself[:], pas[:], adfull[:])
                    nc.vector.scalar_tensor_tensor(
                        tself[:], tself[:], NEG_SLOPE, tself[:],
                        op0=mybir.AluOpType.mult, op1=mybir.AluOpType.max)
                    pselfc = pa2.tile([16, CH], F32, tag="pselfc2")
                    nc.scalar.activation(pselfc[:], tself[:],
                                         mybir.ActivationFunctionType.Exp)
                    nc.scalar.copy(pselfv[:, sl], pselfc[:])
                    nc.vector.tensor_mul(wselfv[:, sl], pselfc[:], h2ch[:])

            run_layer(1, write2)

            # ---------------- log_softmax + transpose + store -------------
            with (
                tc.tile_pool(name="fin", bufs=2) as fin,
                tc.tile_pool(name="finp", bufs=2, space="PSUM") as finp,
                tc.tile_pool(name="fino", bufs=1) as fino,
            ):
                nodemaj = fino.tile([128, NBLK, H], F32, tag="nodemaj")
                for r in range(0, NBLK, 16):
                    nb = min(16, NBLK - r)
                    ptp = finp.tile([128, 16, 16], F32, tag="ptp")
                    for j in range(nb):
                        nc.tensor.transpose(ptp[:, j, :],
                                            out2v[:, (r + j) * 128:(r + j + 1) * 128],
                                            idt[64:80, :])
                    nc.vector.tensor_copy(nodemaj[:, r:r + nb, :], ptp[:, 0:nb, :])
                mx = fin.tile([128, NBLK], F32, tag="mx")
                nc.vector.tensor_reduce(mx[:], nodemaj[:], axis=mybir.AxisListType.X,
                                        op=mybir.AluOpType.max)
                zz = fino.tile([128, NBLK, H], F32, tag="zz")
                nc.vector.tensor_sub(zz[:], nodemaj[:],
                                     mx[:, :, None].to_broadcast([128, NBLK, H]))
                es = fino.tile([128, NBLK, H], F32, tag="es")
                nc.scalar.activation(es[:], zz[:], mybir.ActivationFunctionType.Exp)
                sm = fin.tile([128, NBLK], F32, tag="sm")
                nc.vector.tensor_reduce(sm[:], es[:], axis=mybir.AxisListType.X,
                                        op=mybir.AluOpType.add)
                ls = fin.tile([128, NBLK], F32, tag="ls")
                nc.scalar.activation(ls[:], sm[:], mybir.ActivationFunctionType.Ln)
                outf = fino.tile([128, NBLK, H], F32, tag="outf")
                nc.vector.tensor_sub(outf[:], zz[:],
                                     ls[:, :, None].to_broadcast([128, NBLK, H]))
                nc.gpsimd.dma_start(out=outp[:].rearrange("p (b h) -> p b h", h=H),
                                    in_=outf[:])

    nc.compile()
    if split:
        split_waits(nc, max_waits=max_waits, ctrl_max_waits=ctrl_max_waits)
    return nc


CTRL_TYPES = ("InstDrain", "InstNoOp", "InstHalt", "InstEventSemaphore")


def split_waits(nc, max_waits=2, ctrl_max_waits=1):
    """walrus in this container caps sync-waits per instruction; move excess
    waits onto preceding same-engine NoOps (each carrying one wait)."""
    for f in nc.m.functions:
        for bb in f.blocks:
            new_insts, changed = [], False
            for ins in bb.instructions:
                si = ins.sync_info
                cap = (ctrl_max_waits if type(ins).__name__ in CTRL_TYPES
                       else max_waits)
                if si is not None and si.on_wait is not None and len(si.on_wait) > cap:
                    waits = list(si.on_wait)
                    excess, keep = waits[:-cap] if cap else waits, waits[-cap:] if cap else []
                    for i, w in enumerate(excess):
                        nop = mybir.InstNoOp(name=f"{ins.name}-ws{i}", ins=[], outs=[])
                        nop.engine = ins.engine
                        nop.sync_info = mybir.SyncInfo(on_wait=[w], on_update=[])
                        new_insts.append(nop)
                    si.on_wait = keep
                    changed = True
                new_insts.append(ins)
            if changed:
                bb.instructions = new_insts
    # verify the rewrite stuck (pyo3 lists can copy-on-read)
    for f in nc.m.functions:
        for bb in f.blocks:
            for ins in bb.instructions:
                si = ins.sync_info
                cap = (ctrl_max_waits if type(ins).__name__ in CTRL_TYPES
                       else max_waits)
                assert si is None or si.on_wait is None or len(si.on_wait) <= cap, \
                    f"{ins.name}: {len(si.on_wait)} waits > {cap}"


# ------------------------------------------------------------ input packing

def make_in_maps(inputs, cfg, per_core):
    NC, NL, H, F = cfg["NCORES"], cfg["NL"], cfg["H"], cfg["F"]
    NB, CH, NCH = cfg["NB"], cfg["CH"], cfg["NCH"]
    KB = F // 128
    NLP = NL + 16
    x = np.asarray(inputs["x"], dtype=np.float32)
    sel = np.zeros((128, 16), dtype=np.float32)
    sel[np.arange(128), np.arange(128) % 16] = 1.0
    dstid = np.concatenate(
        [np.arange(NB), np.arange(16)]).astype(np.float32)[None, :]
    shared = {
        "w1": np.ascontiguousarray(np.asarray(inputs["W1"], np.float32)),
        "w2": np.ascontiguousarray(np.asarray(inputs["W2"], np.float32)),
        "a1rep": np.ascontiguousarray(np.repeat(np.asarray(inputs["a_src1"], np.float32)[:, None], 16, 1)),
        "ad1rep": np.ascontiguousarray(np.repeat(np.asarray(inputs["a_dst1"], np.float32)[:, None], 16, 1)),
        "a2rep": np.ascontiguousarray(np.repeat(np.asarray(inputs["a_src2"], np.float32)[:, None], 16, 1)),
        "ad2rep": np.ascontiguousarray(np.repeat(np.asarray(inputs["a_dst2"], np.float32)[:, None], 16, 1)),
        "b1p": np.ascontiguousarray(np.asarray(inputs["b1"], np.float32)[:, None]),
        "b2p": np.ascontiguousarray(np.asarray(inputs["b2"], np.float32)[:, None]),
    }
    shared.update({
        "selp": sel,
        "identp": np.eye(16, dtype=np.float32),
        "dstidp": np.ascontiguousarray(dstid),
    })
    in_maps = []
    for c in range(NC):
        m = dict(shared)
        xpart = x[c * NL:(c + 1) * NL]                    # [NL, F]
        xtw = (xpart.T.reshape(KB, 128, NCH, CH)
               .transpose(1, 2, 0, 3).reshape(128, NCH * KB * CH))
        m["xtw"] = np.ascontiguousarray(xtw)
        m["idxs"] = per_core[c]["idxs"]
        in_maps.append(m)
    return in_maps


def unshard_output(results, cfg):
    NC, NL, H = cfg["NCORES"], cfg["NL"], cfg["H"]
    NBLK = math.ceil(NL / 128)
    parts = []
    for c in range(NC):
        a = np.asarray(results[c]["out"]).reshape(128, NBLK, H)
        a = a.transpose(1, 0, 2).reshape(NBLK * 128, H)[:NL]
        parts.append(a)
    return np.concatenate(parts, axis=0)


# ------------------------------------------------------------------- driver

_CACHE = {}


def run_on_hw(inputs, cfg, trace=False, tmpdir=None):
    import os
    import shutil
    from concourse.bass_utils import run_bass_kernel_spmd
    if tmpdir is not None and os.path.isdir(tmpdir):
        shutil.rmtree(tmpdir, ignore_errors=True)
    if tmpdir is not None:
        os.makedirs(tmpdir, exist_ok=True)
    T_e, per_core = host_prep(inputs["edge_index"], cfg)
    key = (cfg["N"], T_e)
    if key not in _CACHE:
        _CACHE[key] = build_nc(cfg, T_e)
    nc = _CACHE[key]
    in_maps = make_in_maps(inputs, cfg, per_core)
    res = run_bass_kernel_spmd(nc, in_maps, list(range(cfg["NCORES"])),
                               trace=trace, tmpdir=tmpdir)
    out = unshard_output(res.results, cfg)
    return out, res


def kernel(**inputs):
    out, _ = run_on_hw(inputs, FULL_CFG)
    return out.astype(np.float32)



# revision 4
# speedup vs baseline: 1.5405x; 1.5405x over previous
"""GAT (2-layer, heads=1) on 8 Trainium2 NeuronCores.

Strategy (1D node partition):
  - Nodes are split into 8 chunks of NL; core c owns dst-chunk c.
  - Layer tables h^T/alpha_src are computed per-chunk (feature-major via
    PE matmuls on x^T), AllGathered, and kept in SBUF as a per-partition-group
    table: partition group g (16 partitions) holds (h|alpha_src) of chunk g,
    packed two fp16 per fp32 word.
  - Edges are bucketed on the host by (dst-core, src-chunk, dst-block) and
    sorted by dst. Each edge tile covers one block of NB dst nodes for all
    8 src-chunk groups at once; per-edge h|alpha_src gathers run on GPSIMD
    (ap_gather), per-edge math on DVE/ACT, and segment-sums via masked
    tensor_tensor_scan (state = mask*state + w, mask=0 at segment starts)
    + a boundary gather at each node's last edge. Group partials are
    combined with a matmul against a 0/1 selection matrix; softmax
    normalization divides at the node level.
  - Per-edge alpha_dst values are NOT gathered. Instead, for each tile a
    GPSIMD local_scatter places each dst-slot's alpha_dst (fp16) at that
    slot's first-edge stream position (host-known structure; per-partition
    independent indices; ~3.4us per 512-idx scatter vs ~64us for the old
    per-edge ap_gather), and a masked scan broadcasts the value along the
    run. The run-start mask itself is host-precomputed structure, DMA'd
    per tile (HWDGE), freeing GPSIMD entirely.
  - local_scatter and ap_gather live in different GPSIMD IRAM libraries, so
    each layer runs all local_scatters in a "seed phase" (results staged in
    DRAM), then all ap_gathers in the edge phase, with a scheduler fence
    between them -> 2 library reloads per layer instead of per-tile thrash.
  - Slot 0 of every bucket is a sentinel; nodes with no edges in a bucket
    point their boundary index at slot 0 (whose running sum is always 0).
  - Softmax max-subtraction is skipped: attention logits here are O(1), and
    alpha = exp(e)/sum(exp(e)) is shift-invariant.

Host preprocessing only reorders/buckets edge indices and emits 0/1 masks
(structure), never touches float data dependent on device results.
"""

import math
import numpy as np

from concourse import bass, bacc, mybir
import concourse.tile as tile

F32 = mybir.dt.float32
BF16 = mybir.dt.bfloat16
F16 = mybir.dt.float16
I16 = mybir.dt.int16
I32 = mybir.dt.int32

NEG_SLOPE = 0.2
# fp32 word 0xF7500000: low fp16 lane (h) = 0, high fp16 lane
# (alpha_src) = -29952 -> exp(lrelu(...)) == 0, so sentinel edges vanish.
SENT_PACKED = float(
    np.frombuffer(
        (np.uint32(np.float16(-29952.0).view(np.uint16)) << np.uint32(16)).tobytes(),
        np.float32,
    )[0]
)

FULL_CFG = dict(
    NCORES=8, N=100000, F=512, H=16,
    NL=12500, NB=500, NT=25, CH=500, NCH=25,
)

LS_MAX = 2046   # local_scatter num_elems limit (num_elems*32 < 2^16)


def _halves(T_e):
    nh = math.ceil(T_e / LS_MAX)
    w0 = (T_e + nh - 1) // nh
    w0 = (w0 + 1) // 2 * 2
    return nh, w0


# ---------------------------------------------------------------- host prep

def _round_up(x, m):
    return (x + m - 1) // m * m


def host_prep(edge_index, cfg):
    """Bucket and sort edges; build device index streams and masks
    (structure only, no float data).

    Returns (T_e, per_core) where per_core[c] has:
      'idxs'  [128, NT*(T16+32)] i16   (src-local gather idx + boundary idx)
      'maskp' [128, NT*T_e]      f16   (1 inside a run, 0 at run starts)
      'seedp' [128, NT*NH*512]   i16   (run-start positions per dst slot,
                                        split into NH stream halves; -1 = none)
    """
    NC, N, NL = cfg["NCORES"], cfg["N"], cfg["NL"]
    NB, NT = cfg["NB"], cfg["NT"]
    G = 8

    # Self-loops are NOT added here: their contribution is computed
    # analytically at the node level on-device (no gather needed).
    src = np.asarray(edge_index[0], dtype=np.int64)
    dst = np.asarray(edge_index[1], dtype=np.int64)

    core = dst // NL
    grp = src // NL
    order = np.lexsort((src, dst, grp, core))
    src, dst, core, grp = (a[order] for a in (src, dst, core, grp))
    blk = (dst % NL) // NB

    bucket = (core * G + grp) * NT + blk
    nbuck = NC * G * NT
    counts = np.bincount(bucket, minlength=nbuck)
    # +1: slot 0 of every bucket is a sentinel (known-zero running sum)
    T_e = _round_up(int(counts.max()) + 17, 128)
    assert T_e <= 32767
    NH, W0 = _halves(T_e)

    starts = np.zeros(nbuck, dtype=np.int64)
    starts[1:] = np.cumsum(counts)[:-1]
    pos = np.arange(src.size) - starts[bucket] + 1

    is_last = np.ones(src.size, dtype=bool)
    is_last[:-1] = ~((bucket[1:] == bucket[:-1]) & (dst[1:] == dst[:-1]))
    is_first = np.ones(src.size, dtype=bool)
    is_first[1:] = ~((bucket[1:] == bucket[:-1]) & (dst[1:] == dst[:-1]))

    srcl = np.full((NC, G, NT, T_e), NL, dtype=np.int16)
    bnd = np.zeros((NC, G, NT, 512), dtype=np.int16)  # default: sentinel slot 0
    mask = np.ones((NC, G, NT, T_e), dtype=np.float16)
    mask[..., 0] = 0.0
    seed = np.full((NC, G, NT, NH, 512), -1, dtype=np.int16)

    c_, g_, b_ = core, grp, blk
    dloc = (dst % NL) % NB
    srcl[c_, g_, b_, pos] = (src % NL).astype(np.int16)
    bnd[c_[is_last], g_[is_last], b_[is_last],
        dloc[is_last]] = pos[is_last].astype(np.int16)
    mask[c_[is_first], g_[is_first], b_[is_first], pos[is_first]] = 0.0
    p1 = pos[is_first]
    hh = (p1 // W0).astype(np.int64)
    seed[c_[is_first], g_[is_first], b_[is_first], hh,
         dloc[is_first]] = (p1 - hh * W0).astype(np.int16)

    def wrap(a, w):
        # [NC, G, NT, w] -> [NC, 128, NT, w//16]; w16[c, 16g+p, t, s] = a[c, g, t, s*16+p]
        n = a.shape[-1]
        return (a.reshape(NC, G, NT, n // 16, 16)
                 .transpose(0, 1, 4, 2, 3)
                 .reshape(NC, 128, NT, n // 16))

    src_w, bnd_w = wrap(srcl, T_e), wrap(bnd, 512)
    idxs = np.concatenate([src_w, bnd_w], axis=3)  # [NC,128,NT,IW]
    IW = idxs.shape[3]
    idxs = np.ascontiguousarray(idxs.reshape(NC, 128, NT * IW))

    # per-partition replication within each 16-partition group
    maskw = np.ascontiguousarray(
        np.repeat(mask, 16, axis=1).reshape(NC, 128, NT * T_e))
    seedw = np.ascontiguousarray(
        np.repeat(seed.reshape(NC, G, NT, NH * 512), 16, axis=1)
        .reshape(NC, 128, NT * NH * 512))

    per_core = [{"idxs": idxs[c], "maskp": maskw[c], "seedp": seedw[c]}
                for c in range(NC)]
    return T_e, per_core


# ------------------------------------------------------------- device build

def build_nc(cfg, T_e, max_waits=2, ctrl_max_waits=1, split=True):
    NC, N, F, H = cfg["NCORES"], cfg["N"], cfg["F"], cfg["H"]
    NL, NB, NT = cfg["NL"], cfg["NB"], cfg["NT"]
    CH, NCH = cfg["CH"], cfg["NCH"]
    KB = F // 128
    NLP = NL + 16               # table width incl. sentinel
    T16 = T_e // 16
    IW = T16 + 512 // 16
    NH, W0 = _halves(T_e)
    NBLK = math.ceil(NL / 128)
    NL2 = NBLK * 128
    W2 = NL2 // 2               # fp32 words backing the bf16 out1T row
    rg = [list(range(NC))]

    nc = bacc.Bacc("TRN2", target_bir_lowering=False)

    xtw = nc.declare_dram_parameter("xtw", [128, NCH * KB * CH], F32, isOutput=False)
    w1 = nc.declare_dram_parameter("w1", [F, H], F32, isOutput=False)
    w2 = nc.declare_dram_parameter("w2", [H, H], F32, isOutput=False)
    a1rep = nc.declare_dram_parameter("a1rep", [H, 16], F32, isOutput=False)
    ad1rep = nc.declare_dram_parameter("ad1rep", [H, 16], F32, isOutput=False)
    a2rep = nc.declare_dram_parameter("a2rep", [H, 16], F32, isOutput=False)
    ad2rep = nc.declare_dram_parameter("ad2rep", [H, 16], F32, isOutput=False)
    b1p = nc.declare_dram_parameter("b1p", [H, 1], F32, isOutput=False)
    b2p = nc.declare_dram_parameter("b2p", [H, 1], F32, isOutput=False)
    selp = nc.declare_dram_parameter("selp", [128, 16], F32, isOutput=False)
    identp = nc.declare_dram_parameter("identp", [16, 16], F32, isOutput=False)
    idxsp = nc.declare_dram_parameter("idxs", [128, NT * IW], I16, isOutput=False)
    maskp = nc.declare_dram_parameter("maskp", [128, NT * T_e], F16, isOutput=False)
    seedp = nc.declare_dram_parameter("seedp", [128, NT * NH * 512], I16,
                                      isOutput=False)
    outp = nc.declare_dram_parameter("out", [128, NBLK * H], F32, isOutput=True)

    ag_in = [nc.dram_tensor(f"ag_in{l}", [16, NL], F32) for l in (1, 2)]
    ag_out = [nc.dram_tensor(f"ag_out{l}", [128, NL], F32, addr_space="Shared")
              for l in (1, 2)]
    # per-node fp16 alpha_dst rows (own chunk), incl. zero pad wide enough
    # for the last tile's 512-wide adbt broadcast read
    NLP2 = max(NLP, (NT - 1) * NB + 512)
    ad_row = [nc.dram_tensor(f"ad_row{l}", [1, NLP2], F16) for l in (1, 2)]
    # per-edge alpha_dst streams staged by the seed phase
    adeg = [nc.dram_tensor(f"adeg{l}", [128, NT * T_e], F16) for l in (1, 2)]

    with tile.TileContext(nc, num_cores=NC) as tc:
        with tc.tile_pool(name="const", bufs=1) as cpool:
            w1t = cpool.tile([128, KB, H], BF16)
            nc.gpsimd.dma_start(out=w1t[:], in_=w1[:].rearrange("(b p) h -> p b h", p=128))
            # w2 / identity copies aligned to the partitions where out1T /
            # out2T live (matmul needs lhsT and rhs on the same partitions).
            w2t = cpool.tile([48, H], BF16)
            nc.gpsimd.dma_start(out=w2t[32:48, :], in_=w2[:])
            a1t = cpool.tile([16, 16], BF16)
            nc.gpsimd.dma_start(out=a1t[:], in_=a1rep[:])
            ad1t = cpool.tile([16, 16], BF16)
            nc.gpsimd.dma_start(out=ad1t[:], in_=ad1rep[:])
            a2t = cpool.tile([16, 16], BF16)
            nc.gpsimd.dma_start(out=a2t[:], in_=a2rep[:])
            ad2t = cpool.tile([16, 16], BF16)
            nc.gpsimd.dma_start(out=ad2t[:], in_=ad2rep[:])
            b1t = cpool.tile([16, 1], F32)
            nc.sync.dma_start(out=b1t[:], in_=b1p[:])
            b2t = cpool.tile([16, 1], F32)
            nc.sync.dma_start(out=b2t[:], in_=b2p[:])
            selt = cpool.tile([128, 16], F16)
            nc.gpsimd.dma_start(out=selt[:], in_=selp[:])
            idt = cpool.tile([80, 16], F32)
            nc.sync.dma_start(out=idt[64:80, :], in_=identp[:])

            # Stacked per-layer state, one tile so 16-partition rows share
            # column space; engine accesses must start at partition 0/32/64/96:
            # p0-15 wself, p32-47 out1T (bf16, matmul rhs -> base 32),
            # p64-79 out2T (transpose input -> base 64), p96-111 pself.
            stk = cpool.tile([128, NL2], F32)
            wselfv = stk[0:16, 0:NL]
            pselfv = stk[96:112, 0:NL]
            out1v = stk[32:48, 0:W2].bitcast(BF16)      # [16, NL2] bf16
            out2v = stk[64:80, 0:NL2]
            nc.vector.memset(stk[64:80, NL:], 0.0)

            zero16 = cpool.tile([1, NLP2 - NL], F16)
            nc.vector.memset(zero16[:], 0.0)

            idxs_sb = cpool.tile([128, NT * IW], I16)
            nc.gpsimd.dma_start(out=idxs_sb[:], in_=idxsp[:])

            # ---------------- phase A (layer 1): tables from x^T ----------
            with (
                tc.tile_pool(name="pa", bufs=2) as pa,
                tc.tile_pool(name="pap", bufs=2, space="PSUM") as pap,
            ):
                for c in range(NCH):
                    sl = slice(c * CH, (c + 1) * CH)
                    xt_t = pa.tile([128, KB, CH], BF16, tag="xt")
                    nc.gpsimd.dma_start(
                        out=xt_t[:],
                        in_=xtw[:, c * KB * CH:(c + 1) * KB * CH]
                        .rearrange("p (b n) -> p b n", b=KB))
                    ph = pap.tile([16, CH], F32, tag="ph")
                    for b in range(KB):
                        nc.tensor.matmul(ph[:], lhsT=w1t[:, b, :], rhs=xt_t[:, b, :],
                                         start=(b == 0), stop=(b == KB - 1))
                    hch = pa.tile([16, CH], BF16, tag="hch")
                    nc.scalar.copy(hch[:], ph[:])
                    pas = pap.tile([16, CH], F32, tag="pas")
                    nc.tensor.matmul(pas[:], lhsT=a1t[:], rhs=hch[:], start=True, stop=True)
                    pad_ = pap.tile([16, CH], F32, tag="pad")
                    nc.tensor.matmul(pad_[:], lhsT=ad1t[:], rhs=hch[:], start=True, stop=True)
                    packed = pa.tile([16, CH], F32, tag="packed")
                    pk = packed[:].bitcast(F16)
                    nc.vector.tensor_copy(pk[:, 0::2], hch[:])
                    nc.vector.tensor_copy(pk[:, 1::2], pas[:])
                    nc.gpsimd.dma_start(out=ag_in[0][:, sl], in_=packed[:])
                    adrow = pa.tile([1, CH], F16, tag="adrow")
                    nc.vector.tensor_copy(adrow[:], pad_[0:1, :])
                    nc.gpsimd.dma_start(out=ad_row[0][0:1, sl], in_=adrow[:])
                    # analytic self-loop contribution for this chunk
                    adfull = pa.tile([16, CH], F32, tag="adfull")
                    nc.scalar.copy(adfull[:], pad_[:])
                    tself = pa.tile([16, CH], F32, tag="tself")
                    nc.vector.tensor_add(tself[:], pas[:], adfull[:])
                    nc.vector.scalar_tensor_tensor(
                        tself[:], tself[:], NEG_SLOPE, tself[:],
                        op0=mybir.AluOpType.mult, op1=mybir.AluOpType.max)
                    pselfc = pa.tile([16, CH], F32, tag="pselfc")
                    nc.scalar.activation(pselfc[:], tself[:],
                                         mybir.ActivationFunctionType.Exp)
                    nc.scalar.copy(pselfv[:, sl], pselfc[:])
                    nc.vector.tensor_mul(wselfv[:, sl], pselfc[:], hch[:])

            def run_layer(l, writer):
                nc.gpsimd.dma_start(out=ad_row[l][0:1, NL:], in_=zero16[:])
                nc.gpsimd.collective_compute(
                    "AllGather", mybir.AluOpType.bypass, replica_groups=rg,
                    ins=[ag_in[l][:]], outs=[ag_out[l][:]])
                # ---- seed phase: alpha_dst -> run-start seeds -> scan ----
                with tc.tile_pool(name=f"sd{l}", bufs=3) as sd:
                    for t in range(NT):
                        adbt = sd.tile([128, 512], F16, tag="adbt")
                        nc.gpsimd.dma_start(
                            out=adbt[:],
                            in_=ad_row[l][0:1, t * NB:t * NB + 512]
                            .to_broadcast([128, 512]))
                        sidx = sd.tile([128, NH * 512], I16, tag="sidx")
                        nc.sync.dma_start(
                            out=sidx[:],
                            in_=seedp[:, t * NH * 512:(t + 1) * NH * 512])
                        seeds = sd.tile([128, T_e], F16, tag="seeds")
                        for h in range(NH):
                            w0, w1_ = h * W0, min((h + 1) * W0, T_e)
                            nc.gpsimd.local_scatter(
                                seeds[:, w0:w1_], adbt[:],
                                sidx[:, h * 512:(h + 1) * 512],
                                channels=128, num_elems=w1_ - w0, num_idxs=512)
                        msk = sd.tile([128, T_e], F16, tag="msk")
                        nc.sync.dma_start(
                            out=msk[:], in_=maskp[:, t * T_e:(t + 1) * T_e])
                        ade = sd.tile([128, T_e], F16, tag="ade")
                        nc.vector.tensor_tensor_scan(
                            ade[:], msk[:], seeds[:], 0.0,
                            mybir.AluOpType.mult, mybir.AluOpType.add)
                        nc.sync.dma_start(
                            out=adeg[l][:, t * T_e:(t + 1) * T_e], in_=ade[:])
                tc.no_sync_barrier()
                # ---- edge phase: h|as gather + per-edge math + seg-sums ----
                with tc.tile_pool(name=f"tab{l}", bufs=1) as tabp:
                    table = tabp.tile([128, NLP], F32, tag="table")
                    nc.gpsimd.dma_start(out=table[:, :NL], in_=ag_out[l][:])
                    nc.vector.memset(table[:, NL:], SENT_PACKED)
                    with (
                        tc.tile_pool(name=f"ed{l}", bufs=2) as ed,
                        tc.tile_pool(name=f"eb{l}", bufs=3) as eb,
                        tc.tile_pool(name=f"e1{l}", bufs=1) as e1,
                        tc.tile_pool(name=f"edp{l}", bufs=2, space="PSUM") as edp,
                    ):
                        for t in range(NT):
                            o = t * IW
                            i_src = idxs_sb[:, o:o + T16]
                            i_bnd = idxs_sb[:, o + T16:o + IW]
                            ghs = ed.tile([128, T_e], F32, tag="ghs")
                            nc.gpsimd.ap_gather(
                                ghs[:], table[:], i_src,
                                channels=128, num_elems=NLP, d=1, num_idxs=T_e)
                            ade = e1.tile([128, T_e], F16, tag="adg", bufs=2)
                            nc.sync.dma_start(
                                out=ade[:], in_=adeg[l][:, t * T_e:(t + 1) * T_e])
                            msk = e1.tile([128, T_e], F16, tag="mask", bufs=2)
                            nc.sync.dma_start(
                                out=msk[:], in_=maskp[:, t * T_e:(t + 1) * T_e])
                            g16 = ghs[:].bitcast(F16)   # [128, 2*T_e]
                            tt = ed.tile([128, T_e], F16, tag="tt")
                            nc.vector.tensor_add(tt[:], g16[:, 1::2], ade[:])
                            nc.vector.scalar_tensor_tensor(
                                tt[:], tt[:], NEG_SLOPE, tt[:],
                                op0=mybir.AluOpType.mult, op1=mybir.AluOpType.max)
                            pp = e1.tile([128, T_e], F16, tag="pp")
                            nc.scalar.activation(pp[:], tt[:],
                                                 mybir.ActivationFunctionType.Exp)
                            ww = ed.tile([128, T_e], F16, tag="ww")
                            nc.vector.tensor_mul(ww[:], pp[:], g16[:, 0::2])
                            sc = eb.tile([128, T_e], F32, tag="sc", bufs=1)
                            sc16 = sc[:].bitcast(F16)   # (w | p) lanes
                            nc.vector.tensor_tensor_scan(
                                sc16[:, 0::2], msk[:], ww[:], 0.0,
                                mybir.AluOpType.mult, mybir.AluOpType.add)
                            nc.vector.tensor_tensor_scan(
                                sc16[:, 1::2], msk[:], pp[:], 0.0,
                                mybir.AluOpType.mult, mybir.AluOpType.add)
                            bg = e1.tile([128, 512], F32, tag="bg")
                            nc.gpsimd.ap_gather(
                                bg[:], sc[:], i_bnd,
                                channels=128, num_elems=T_e, d=1, num_idxs=512)
                            bg16 = bg[:].bitcast(F16)
                            pu = edp.tile([16, 512], F32, tag="pu")
                            nc.tensor.matmul(pu[:], lhsT=selt[:], rhs=bg16[:, 0::2],
                                             start=True, stop=True)
                            ps = edp.tile([16, 512], F32, tag="ps")
                            nc.tensor.matmul(ps[:], lhsT=selt[:], rhs=bg16[:, 1::2],
                                             start=True, stop=True)
                            # add analytic self-loop terms, then normalize
                            den = ed.tile([16, NB], F32, tag="s16", bufs=4)
                            nc.vector.tensor_add(den[:], ps[:, :NB],
                                                 pselfv[:, t * NB:(t + 1) * NB])
                            rc = ed.tile([16, NB], F32, tag="s16", bufs=4)
                            nc.vector.reciprocal_approx_fast(out=rc[:], in_=den[:])
                            num = ed.tile([16, NB], F32, tag="s16", bufs=4)
                            nc.vector.tensor_add(num[:], pu[:, :NB],
                                                 wselfv[:, t * NB:(t + 1) * NB])
                            uv = ed.tile([16, NB], F32, tag="s16", bufs=4)
                            nc.vector.tensor_mul(uv[:], num[:], rc[:])
                            writer(t, uv)

            def write1(t, uv):
                nc.scalar.activation(out1v[:, t * NB:(t + 1) * NB], uv[:],
                                     mybir.ActivationFunctionType.Relu,
                                     bias=b1t[:, 0:1])

            def write2(t, uv):
                nc.vector.tensor_scalar_add(out2v[:, t * NB:(t + 1) * NB], uv[:],
                                            b2t[:, 0:1])

            run_layer(0, write1)

            # ---------------- phase A (layer 2): tables from out1T --------
            with (
                tc.tile_pool(name="pa2", bufs=2) as pa2,
                tc.tile_pool(name="pap2", bufs=2, space="PSUM") as pap2,
            ):
                for c in range(NCH):
                    sl = slice(c * CH, (c + 1) * CH)
                    ph = pap2.tile([16, CH], F32, tag="ph2")
                    nc.tensor.matmul(ph[:], lhsT=w2t[32:48, :], rhs=out1v[:, sl],
                                     start=True, stop=True)
                    h2ch = pa2.tile([16, CH], BF16, tag="h2ch")
                    nc.scalar.copy(h2ch[:], ph[:])
                    pas = pap2.tile([16, CH], F32, tag="pas2")
                    nc.tensor.matmul(pas[:], lhsT=a2t[:], rhs=h2ch[:], start=True, stop=True)
                    pad_ = pap2.tile([16, CH], F32, tag="pad2")
                    nc.tensor.matmul(pad_[:], lhsT=ad2t[:], rhs=h2ch[:], start=True, stop=True)
                    packed = pa2.tile([16, CH], F32, tag="packed2")
                    pk = packed[:].bitcast(F16)
                    nc.vector.tensor_copy(pk[:, 0::2], h2ch[:])
                    nc.vector.tensor_copy(pk[:, 1::2], pas[:])
                    nc.gpsimd.dma_start(out=ag_in[1][:, sl], in_=packed[:])
                    adrow = pa2.tile([1, CH], F16, tag="adrow2")
                    nc.vector.tensor_copy(adrow[:], pad_[0:1, :])
                    nc.gpsimd.dma_start(out=ad_row[1][0:1, sl], in_=adrow[:])
                    adfull = pa2.tile([16, CH], F32, tag="adfull2")
                    nc.scalar.copy(adfull[:], pad_[:])
                    tself = pa2.tile([16, CH], F32, tag="tself2")
                    nc.vector.tensor_add(tself[:], pas[:], adfull[:])
                    nc.vector.scalar_tensor_tensor(
                        tself[:], tself[:], NEG_SLOPE, tself[:],
                        op0=mybir.AluOpType.mult, op1=mybir.AluOpType.max)
                    pselfc = pa2.tile([16, CH], F32, tag="pselfc2")
                    nc.scalar.activation(pselfc[:], tself[:],
                                         mybir.ActivationFunctionType.Exp)
                    nc.scalar.copy(pselfv[:, sl], pselfc[:])
                    nc.vector.tensor_mul(wselfv[:, sl], pselfc[:], h2ch[:])

            run_layer(1, write2)

            # ---------------- log_softmax + transpose + store -------------
            with (
                tc.tile_pool(name="fin", bufs=2) as fin,
                tc.tile_pool(name="finp", bufs=2, space="PSUM") as finp,
                tc.tile_pool(name="fino", bufs=1) as fino,
            ):
                nodemaj = fino.tile([128, NBLK, H], F32, tag="nodemaj")
                for r in range(0, NBLK, 16):
                    nb = min(16, NBLK - r)
                    ptp = finp.tile([128, 16, 16], F32, tag="ptp")
                    for j in range(nb):
                        nc.tensor.transpose(ptp[:, j, :],
                                            out2v[:, (r + j) * 128:(r + j + 1) * 128],
                                            idt[64:80, :])
                    nc.vector.tensor_copy(nodemaj[:, r:r + nb, :], ptp[:, 0:nb, :])
                mx = fin.tile([128, NBLK], F32, tag="mx")
                nc.vector.tensor_reduce(mx[:], nodemaj[:], axis=mybir.AxisListType.X,
                                        op=mybir.AluOpType.max)
                zz = fino.tile([128, NBLK, H], F32, tag="zz")
                nc.vector.tensor_sub(zz[:], nodemaj[:],
                                     mx[:, :, None].to_broadcast([128, NBLK, H]))
                es = fino.tile([128, NBLK, H], F32, tag="es")
                nc.scalar.activation(es[:], zz[:], mybir.ActivationFunctionType.Exp)
                sm = fin.tile([128, NBLK], F32, tag="sm")
                nc.vector.tensor_reduce(sm[:], es[:], axis=mybir.AxisListType.X,
                                        op=mybir.AluOpType.add)
                ls = fin.tile([128, NBLK], F32, tag="ls")
                nc.scalar.activation(ls[:], sm[:], mybir.ActivationFunctionType.Ln)
                outf = fino.tile([128, NBLK, H], F32, tag="outf")
                nc.vector.tensor_sub(outf[:], zz[:],
                                     ls[:, :, None].to_broadcast([128, NBLK, H]))
                nc.gpsimd.dma_start(out=outp[:].rearrange("p (b h) -> p b h", h=H),
                                    in_=outf[:])

    nc.compile()
    n_reload = sum(
        1
        for f in nc.m.functions
        for bb in f.blocks
        for ins in bb.instructions
        if type(ins).__name__ == "InstPseudoReloadLibraryIndex"
    )
    assert n_reload <= 8, f"library thrash: {n_reload} reloads"
    if split:
        split_waits(nc, max_waits=max_waits, ctrl_max_waits=ctrl_max_waits)
    return nc


CTRL_TYPES = ("InstDrain", "InstNoOp", "InstHalt", "InstEventSemaphore")


def split_waits(nc, max_waits=2, ctrl_max_waits=1):
    """walrus in this container caps sync-waits per instruction; move excess
    waits onto preceding same-engine NoOps (each carrying one wait)."""
    for f in nc.m.functions:
        for bb in f.blocks:
            new_insts, changed = [], False
            for ins in bb.instructions:
                si = ins.sync_info
                cap = (ctrl_max_waits if type(ins).__name__ in CTRL_TYPES
                       else max_waits)
                if si is not None and si.on_wait is not None and len(si.on_wait) > cap:
                    waits = list(si.on_wait)
                    excess, keep = waits[:-cap] if cap else waits, waits[-cap:] if cap else []
                    for i, w in enumerate(excess):
                        nop = mybir.InstNoOp(name=f"{ins.name}-ws{i}", ins=[], outs=[])
                        nop.engine = ins.engine
                        nop.sync_info = mybir.SyncInfo(on_wait=[w], on_update=[])
                        new_insts.append(nop)
                    si.on_wait = keep
                    changed = True
                new_insts.append(ins)
            if changed:
                bb.instructions = new_insts
    # verify the rewrite stuck (pyo3 lists can copy-on-read)
    for f in nc.m.functions:
        for bb in f.blocks:
            for ins in bb.instructions:
                si = ins.sync_info
                cap = (ctrl_max_waits if type(ins).__name__ in CTRL_TYPES
                       else max_waits)
                assert si is None or si.on_wait is None or len(si.on_wait) <= cap, \
                    f"{ins.name}: {len(si.on_wait)} waits > {cap}"


# ------------------------------------------------------------ input packing

def make_in_maps(inputs, cfg, per_core):
    NC, NL, H, F = cfg["NCORES"], cfg["NL"], cfg["H"], cfg["F"]
    NB, CH, NCH = cfg["NB"], cfg["CH"], cfg["NCH"]
    KB = F // 128
    x = np.asarray(inputs["x"], dtype=np.float32)
    sel = np.zeros((128, 16), dtype=np.float32)
    sel[np.arange(128), np.arange(128) % 16] = 1.0
    shared = {
        "w1": np.ascontiguousarray(np.asarray(inputs["W1"], np.float32)),
        "w2": np.ascontiguousarray(np.asarray(inputs["W2"], np.float32)),
        "a1rep": np.ascontiguousarray(np.repeat(np.asarray(inputs["a_src1"], np.float32)[:, None], 16, 1)),
        "ad1rep": np.ascontiguousarray(np.repeat(np.asarray(inputs["a_dst1"], np.float32)[:, None], 16, 1)),
        "a2rep": np.ascontiguousarray(np.repeat(np.asarray(inputs["a_src2"], np.float32)[:, None], 16, 1)),
        "ad2rep": np.ascontiguousarray(np.repeat(np.asarray(inputs["a_dst2"], np.float32)[:, None], 16, 1)),
        "b1p": np.ascontiguousarray(np.asarray(inputs["b1"], np.float32)[:, None]),
        "b2p": np.ascontiguousarray(np.asarray(inputs["b2"], np.float32)[:, None]),
    }
    shared.update({
        "selp": sel,
        "identp": np.eye(16, dtype=np.float32),
    })
    in_maps = []
    for c in range(NC):
        m = dict(shared)
        xpart = x[c * NL:(c + 1) * NL]                    # [NL, F]
        xtw = (xpart.T.reshape(KB, 128, NCH, CH)
               .transpose(1, 2, 0, 3).reshape(128, NCH * KB * CH))
        m["xtw"] = np.ascontiguousarray(xtw)
        m["idxs"] = per_core[c]["idxs"]
        m["maskp"] = per_core[c]["maskp"]
        m["seedp"] = per_core[c]["seedp"]
        in_maps.append(m)
    return in_maps


def unshard_output(results, cfg):
    NC, NL, H = cfg["NCORES"], cfg["NL"], cfg["H"]
    NBLK = math.ceil(NL / 128)
    parts = []
    for c in range(NC):
        a = np.asarray(results[c]["out"]).reshape(128, NBLK, H)
        a = a.transpose(1, 0, 2).reshape(NBLK * 128, H)[:NL]
        parts.append(a)
    return np.concatenate(parts, axis=0)


# ------------------------------------------------------------------- driver

_CACHE = {}


def run_on_hw(inputs, cfg, trace=False, tmpdir=None):
    import os
    import shutil
    from concourse.bass_utils import run_bass_kernel_spmd
    if tmpdir is not None and os.path.isdir(tmpdir):
        shutil.rmtree(tmpdir, ignore_errors=True)
    if tmpdir is not None:
        os.makedirs(tmpdir, exist_ok=True)
    T_e, per_core = host_prep(inputs["edge_index"], cfg)
    key = (cfg["N"], T_e)
    if key not in _CACHE:
        _CACHE[key] = build_nc(cfg, T_e)
    nc = _CACHE[key]
    in_maps = make_in_maps(inputs, cfg, per_core)
    res = run_bass_kernel_spmd(nc, in_maps, list(range(cfg["NCORES"])),
                               trace=trace, tmpdir=tmpdir)
    out = unshard_output(res.results, cfg)
    return out, res


def kernel(**inputs):
    out, _ = run_on_hw(inputs, FULL_CFG)
    return out.astype(np.float32)


# revision 5
# speedup vs baseline: 1.8250x; 1.1847x over previous
"""GAT (2-layer, heads=1) on 8 Trainium2 NeuronCores.

Strategy (1D node partition):
  - Nodes are split into 8 chunks of NL; core c owns dst-chunk c.
  - Layer tables h^T/alpha_src are computed per-chunk (feature-major via
    PE matmuls on x^T), AllGathered, and kept in SBUF as a per-partition-group
    table: partition group g (16 partitions) holds (h|alpha_src) of chunk g,
    packed two fp16 per fp32 word.
  - Edges are bucketed on the host by (dst-core, src-chunk, dst-block) and
    sorted by dst. Each edge tile covers one block of NB dst nodes for all
    8 src-chunk groups at once; per-edge h|alpha_src gathers run on GPSIMD
    (ap_gather), per-edge math on DVE/ACT, and segment-sums via masked
    tensor_tensor_scan (state = mask*state + w, mask=0 at segment starts)
    + a boundary gather at each node's last edge. Group partials are
    combined with a matmul against a 0/1 selection matrix; softmax
    normalization divides at the node level.
  - Per-edge alpha_dst values are NOT gathered. Instead, for each tile a
    GPSIMD local_scatter places each dst-slot's alpha_dst (fp16) at that
    slot's first-edge stream position (host-known structure; per-partition
    independent indices; ~3.4us per 512-idx scatter vs ~64us for the old
    per-edge ap_gather), and a masked scan broadcasts the value along the
    run. The run-start mask itself is host-precomputed structure, DMA'd
    per tile (HWDGE), freeing GPSIMD entirely.
  - local_scatter and ap_gather live in different GPSIMD IRAM libraries, so
    each layer runs all local_scatters in a "seed phase" (results staged in
    DRAM), then all ap_gathers in the edge phase, with a scheduler fence
    between them -> 2 library reloads per layer instead of per-tile thrash.
  - Slot 0 of every bucket is a sentinel; nodes with no edges in a bucket
    point their boundary index at slot 0 (whose running sum is always 0).
  - Softmax max-subtraction is skipped: attention logits here are O(1), and
    alpha = exp(e)/sum(exp(e)) is shift-invariant.

Host preprocessing only reorders/buckets edge indices and emits 0/1 masks
(structure), never touches float data dependent on device results.
"""

import math
import numpy as np

from concourse import bass, bacc, mybir
import concourse.tile as tile

F32 = mybir.dt.float32
BF16 = mybir.dt.bfloat16
F16 = mybir.dt.float16
I16 = mybir.dt.int16
I32 = mybir.dt.int32

NEG_SLOPE = 0.2
# fp32 word 0xF7500000: low fp16 lane (h) = 0, high fp16 lane
# (alpha_src) = -29952 -> exp(lrelu(...)) == 0, so sentinel edges vanish.
SENT_PACKED = float(
    np.frombuffer(
        (np.uint32(np.float16(-29952.0).view(np.uint16)) << np.uint32(16)).tobytes(),
        np.float32,
    )[0]
)

FULL_CFG = dict(
    NCORES=8, N=100000, F=512, H=16,
    NL=12500, NB=500, NT=25, CH=500, NCH=25,
)

LS_MAX = 2046   # local_scatter num_elems limit (num_elems*32 < 2^16)


def _halves(T_e):
    nh = math.ceil(T_e / LS_MAX)
    w0 = (T_e + nh - 1) // nh
    w0 = (w0 + 1) // 2 * 2
    return nh, w0


# ---------------------------------------------------------------- host prep

def _round_up(x, m):
    return (x + m - 1) // m * m


def host_prep(edge_index, cfg):
    """Bucket and sort edges; build device index streams and masks
    (structure only, no float data).

    Returns (T_e, per_core) where per_core[c] has:
      'idxs'  [128, NT*(T16+32)] i16   (src-local gather idx + boundary idx)
      'maskp' [128, NT*T_e]      f16   (1 inside a run, 0 at run starts)
      'seedp' [128, NT*NH*512]   i16   (run-start positions per dst slot,
                                        split into NH stream halves; -1 = none)
    """
    NC, N, NL = cfg["NCORES"], cfg["N"], cfg["NL"]
    NB, NT = cfg["NB"], cfg["NT"]
    G = 8

    # Self-loops are NOT added here: their contribution is computed
    # analytically at the node level on-device (no gather needed).
    src = np.asarray(edge_index[0], dtype=np.int64)
    dst = np.asarray(edge_index[1], dtype=np.int64)

    core = dst // NL
    grp = src // NL
    order = np.lexsort((src, dst, grp, core))
    src, dst, core, grp = (a[order] for a in (src, dst, core, grp))
    blk = (dst % NL) // NB

    bucket = (core * G + grp) * NT + blk
    nbuck = NC * G * NT
    counts = np.bincount(bucket, minlength=nbuck)
    # +1: slot 0 of every bucket is a sentinel (known-zero running sum)
    T_e = _round_up(int(counts.max()) + 17, 128)
    assert T_e <= 32767
    NH, W0 = _halves(T_e)

    starts = np.zeros(nbuck, dtype=np.int64)
    starts[1:] = np.cumsum(counts)[:-1]
    pos = np.arange(src.size) - starts[bucket] + 1

    is_last = np.ones(src.size, dtype=bool)
    is_last[:-1] = ~((bucket[1:] == bucket[:-1]) & (dst[1:] == dst[:-1]))
    is_first = np.ones(src.size, dtype=bool)
    is_first[1:] = ~((bucket[1:] == bucket[:-1]) & (dst[1:] == dst[:-1]))

    srcl = np.full((NC, G, NT, T_e), NL, dtype=np.int16)
    bnd = np.zeros((NC, G, NT, 512), dtype=np.int16)  # default: sentinel slot 0
    mask = np.ones((NC, G, NT, T_e), dtype=np.float16)
    mask[..., 0] = 0.0
    seed = np.full((NC, G, NT, NH, 512), -1, dtype=np.int16)

    c_, g_, b_ = core, grp, blk
    dloc = (dst % NL) % NB
    srcl[c_, g_, b_, pos] = (src % NL).astype(np.int16)
    bnd[c_[is_last], g_[is_last], b_[is_last],
        dloc[is_last]] = pos[is_last].astype(np.int16)
    mask[c_[is_first], g_[is_first], b_[is_first], pos[is_first]] = 0.0
    p1 = pos[is_first]
    hh = (p1 // W0).astype(np.int64)
    seed[c_[is_first], g_[is_first], b_[is_first], hh,
         dloc[is_first]] = (p1 - hh * W0).astype(np.int16)

    def wrap(a, w):
        # [NC, G, NT, w] -> [NC, 128, NT, w//16]; w16[c, 16g+p, t, s] = a[c, g, t, s*16+p]
        n = a.shape[-1]
        return (a.reshape(NC, G, NT, n // 16, 16)
                 .transpose(0, 1, 4, 2, 3)
                 .reshape(NC, 128, NT, n // 16))

    src_w, bnd_w = wrap(srcl, T_e), wrap(bnd, 512)
    idxs = np.concatenate([src_w, bnd_w], axis=3)  # [NC,128,NT,IW]
    IW = idxs.shape[3]
    idxs = np.ascontiguousarray(idxs.reshape(NC, 128, NT * IW))

    # per-partition replication within each 16-partition group
    maskw = np.ascontiguousarray(
        np.repeat(mask, 16, axis=1).reshape(NC, 128, NT * T_e))
    seedw = np.ascontiguousarray(
        np.repeat(seed.reshape(NC, G, NT, NH * 512), 16, axis=1)
        .reshape(NC, 128, NT * NH * 512))

    per_core = [{"idxs": idxs[c], "maskp": maskw[c], "seedp": seedw[c]}
                for c in range(NC)]
    return T_e, per_core


# ------------------------------------------------------------- device build

def build_nc(cfg, T_e, max_waits=2, ctrl_max_waits=1, split=True):
    NC, N, F, H = cfg["NCORES"], cfg["N"], cfg["F"], cfg["H"]
    NL, NB, NT = cfg["NL"], cfg["NB"], cfg["NT"]
    CH, NCH = cfg["CH"], cfg["NCH"]
    KB = F // 128
    NLP = NL + 16               # table width incl. sentinel
    T16 = T_e // 16
    IW = T16 + 512 // 16
    NH, W0 = _halves(T_e)
    NBLK = math.ceil(NL / 128)
    NL2 = NBLK * 128
    W2 = NL2 // 2               # fp32 words backing the bf16 out1T row
    rg = [list(range(NC))]

    nc = bacc.Bacc("TRN2", target_bir_lowering=False)

    xtw = nc.declare_dram_parameter("xtw", [128, NCH * KB * CH], F32, isOutput=False)
    w1 = nc.declare_dram_parameter("w1", [F, H], F32, isOutput=False)
    w2 = nc.declare_dram_parameter("w2", [H, H], F32, isOutput=False)
    a1rep = nc.declare_dram_parameter("a1rep", [H, 16], F32, isOutput=False)
    ad1rep = nc.declare_dram_parameter("ad1rep", [H, 16], F32, isOutput=False)
    a2rep = nc.declare_dram_parameter("a2rep", [H, 16], F32, isOutput=False)
    ad2rep = nc.declare_dram_parameter("ad2rep", [H, 16], F32, isOutput=False)
    b1p = nc.declare_dram_parameter("b1p", [H, 1], F32, isOutput=False)
    b2p = nc.declare_dram_parameter("b2p", [H, 1], F32, isOutput=False)
    selp = nc.declare_dram_parameter("selp", [128, 16], F32, isOutput=False)
    identp = nc.declare_dram_parameter("identp", [16, 16], F32, isOutput=False)
    idxsp = nc.declare_dram_parameter("idxs", [128, NT * IW], I16, isOutput=False)
    maskp = nc.declare_dram_parameter("maskp", [128, NT * T_e], F16, isOutput=False)
    seedp = nc.declare_dram_parameter("seedp", [128, NT * NH * 512], I16,
                                      isOutput=False)
    outp = nc.declare_dram_parameter("out", [128, NBLK * H], F32, isOutput=True)

    ag_in = [nc.dram_tensor(f"ag_in{l}", [16, NL], F32) for l in (1, 2)]
    ag_out = [nc.dram_tensor(f"ag_out{l}", [128, NL], F32, addr_space="Shared")
              for l in (1, 2)]
    # per-node fp16 alpha_dst rows (own chunk), incl. zero pad wide enough
    # for the last tile's 512-wide adbt broadcast read
    NLP2 = max(NLP, (NT - 1) * NB + 512)
    ad_row = [nc.dram_tensor(f"ad_row{l}", [1, NLP2], F16) for l in (1, 2)]
    # per-edge alpha_dst streams staged by the seed phase
    adeg = [nc.dram_tensor(f"adeg{l}", [128, NT * T_e], F16) for l in (1, 2)]

    with tile.TileContext(nc, num_cores=NC) as tc:
        with tc.tile_pool(name="const", bufs=1) as cpool:
            w1t = cpool.tile([128, KB, H], BF16)
            nc.gpsimd.dma_start(out=w1t[:], in_=w1[:].rearrange("(b p) h -> p b h", p=128))
            # w2 / identity copies aligned to the partitions where out1T /
            # out2T live (matmul needs lhsT and rhs on the same partitions).
            w2t = cpool.tile([48, H], BF16)
            nc.gpsimd.dma_start(out=w2t[32:48, :], in_=w2[:])
            a1t = cpool.tile([16, 16], BF16)
            nc.gpsimd.dma_start(out=a1t[:], in_=a1rep[:])
            ad1t = cpool.tile([16, 16], BF16)
            nc.gpsimd.dma_start(out=ad1t[:], in_=ad1rep[:])
            a2t = cpool.tile([16, 16], BF16)
            nc.gpsimd.dma_start(out=a2t[:], in_=a2rep[:])
            ad2t = cpool.tile([16, 16], BF16)
            nc.gpsimd.dma_start(out=ad2t[:], in_=ad2rep[:])
            b1t = cpool.tile([16, 1], F32)
            nc.sync.dma_start(out=b1t[:], in_=b1p[:])
            b2t = cpool.tile([16, 1], F32)
            nc.sync.dma_start(out=b2t[:], in_=b2p[:])
            selt = cpool.tile([128, 16], F16)
            nc.gpsimd.dma_start(out=selt[:], in_=selp[:])
            idt = cpool.tile([80, 16], F32)
            nc.sync.dma_start(out=idt[64:80, :], in_=identp[:])

            # Stacked per-layer state, one tile so 16-partition rows share
            # column space; engine accesses must start at partition 0/32/64/96:
            # p0-15 wself, p32-47 out1T (bf16, matmul rhs -> base 32),
            # p64-79 out2T (transpose input -> base 64), p96-111 pself.
            stk = cpool.tile([128, NL2], F32)
            wselfv = stk[0:16, 0:NL]
            pselfv = stk[96:112, 0:NL]
            out1v = stk[32:48, 0:W2].bitcast(BF16)      # [16, NL2] bf16
            out2v = stk[64:80, 0:NL2]
            nc.vector.memset(stk[64:80, NL:], 0.0)

            zero16 = cpool.tile([1, NLP2 - NL], F16)
            nc.vector.memset(zero16[:], 0.0)

            idxs_sb = cpool.tile([128, NT * IW], I16)
            nc.gpsimd.dma_start(out=idxs_sb[:], in_=idxsp[:])

            # ---------------- phase A (layer 1): tables from x^T ----------
            with (
                tc.tile_pool(name="pa", bufs=2) as pa,
                tc.tile_pool(name="pap", bufs=2, space="PSUM") as pap,
            ):
                for c in range(NCH):
                    sl = slice(c * CH, (c + 1) * CH)
                    xt_t = pa.tile([128, KB, CH], BF16, tag="xt")
                    nc.gpsimd.dma_start(
                        out=xt_t[:],
                        in_=xtw[:, c * KB * CH:(c + 1) * KB * CH]
                        .rearrange("p (b n) -> p b n", b=KB))
                    ph = pap.tile([16, CH], F32, tag="ph")
                    for b in range(KB):
                        nc.tensor.matmul(ph[:], lhsT=w1t[:, b, :], rhs=xt_t[:, b, :],
                                         start=(b == 0), stop=(b == KB - 1))
                    hch = pa.tile([16, CH], BF16, tag="hch")
                    nc.scalar.copy(hch[:], ph[:])
                    pas = pap.tile([16, CH], F32, tag="pas")
                    nc.tensor.matmul(pas[:], lhsT=a1t[:], rhs=hch[:], start=True, stop=True)
                    pad_ = pap.tile([16, CH], F32, tag="pad")
                    nc.tensor.matmul(pad_[:], lhsT=ad1t[:], rhs=hch[:], start=True, stop=True)
                    packed = pa.tile([16, CH], F32, tag="packed")
                    pk = packed[:].bitcast(F16)
                    nc.vector.tensor_copy(pk[:, 0::2], hch[:])
                    nc.vector.tensor_copy(pk[:, 1::2], pas[:])
                    nc.gpsimd.dma_start(out=ag_in[0][:, sl], in_=packed[:])
                    adrow = pa.tile([1, CH], F16, tag="adrow")
                    nc.vector.tensor_copy(adrow[:], pad_[0:1, :])
                    nc.gpsimd.dma_start(out=ad_row[0][0:1, sl], in_=adrow[:])
                    # analytic self-loop contribution for this chunk
                    adfull = pa.tile([16, CH], F32, tag="adfull")
                    nc.scalar.copy(adfull[:], pad_[:])
                    tself = pa.tile([16, CH], F32, tag="tself")
                    nc.vector.tensor_add(tself[:], pas[:], adfull[:])
                    nc.vector.scalar_tensor_tensor(
                        tself[:], tself[:], NEG_SLOPE, tself[:],
                        op0=mybir.AluOpType.mult, op1=mybir.AluOpType.max)
                    pselfc = pa.tile([16, CH], F32, tag="pselfc")
                    nc.scalar.activation(pselfc[:], tself[:],
                                         mybir.ActivationFunctionType.Exp)
                    nc.scalar.copy(pselfv[:, sl], pselfc[:])
                    nc.vector.tensor_mul(wselfv[:, sl], pselfc[:], hch[:])

            def run_layer(l, writer):
                nc.gpsimd.dma_start(out=ad_row[l][0:1, NL:], in_=zero16[:])
                nc.gpsimd.collective_compute(
                    "AllGather", mybir.AluOpType.bypass, replica_groups=rg,
                    ins=[ag_in[l][:]], outs=[ag_out[l][:]])
                # ---- seed phase: alpha_dst -> run-start seeds -> scan ----
                with tc.tile_pool(name=f"sd{l}", bufs=3) as sd:
                    for t in range(NT):
                        adbt = sd.tile([128, 512], F16, tag="adbt")
                        nc.gpsimd.dma_start(
                            out=adbt[:],
                            in_=ad_row[l][0:1, t * NB:t * NB + 512]
                            .to_broadcast([128, 512]))
                        sidx = sd.tile([128, NH * 512], I16, tag="sidx")
                        nc.sync.dma_start(
                            out=sidx[:],
                            in_=seedp[:, t * NH * 512:(t + 1) * NH * 512])
                        seeds = sd.tile([128, T_e], F16, tag="seeds")
                        for h in range(NH):
                            w0, w1_ = h * W0, min((h + 1) * W0, T_e)
                            nc.gpsimd.local_scatter(
                                seeds[:, w0:w1_], adbt[:],
                                sidx[:, h * 512:(h + 1) * 512],
                                channels=128, num_elems=w1_ - w0, num_idxs=512)
                        msk = sd.tile([128, T_e], F16, tag="msk")
                        nc.sync.dma_start(
                            out=msk[:], in_=maskp[:, t * T_e:(t + 1) * T_e])
                        ade = sd.tile([128, T_e], F16, tag="ade")
                        nc.vector.tensor_tensor_scan(
                            ade[:], msk[:], seeds[:], 0.0,
                            mybir.AluOpType.mult, mybir.AluOpType.add)
                        nc.sync.dma_start(
                            out=adeg[l][:, t * T_e:(t + 1) * T_e], in_=ade[:])
                tc.no_sync_barrier()
                # ---- edge phase: h|as gather + per-edge math + seg-sums ----
                with tc.tile_pool(name=f"tab{l}", bufs=1) as tabp:
                    table = tabp.tile([128, NLP], F32, tag="table")
                    nc.gpsimd.dma_start(out=table[:, :NL], in_=ag_out[l][:])
                    nc.vector.memset(table[:, NL:], SENT_PACKED)
                    with (
                        tc.tile_pool(name=f"ed{l}", bufs=2) as ed,
                        tc.tile_pool(name=f"eb{l}", bufs=3) as eb,
                        tc.tile_pool(name=f"e1{l}", bufs=1) as e1,
                        tc.tile_pool(name=f"edp{l}", bufs=2, space="PSUM") as edp,
                    ):
                        # software-pipelined: issue tile t's h|as gather one
                        # iteration ahead of its per-edge math, so the DVE
                        # chain of tile t hides under the gather of t+1 and
                        # GPSIMD stays busy back-to-back.
                        pend = {}

                        def issue(t):
                            o = t * IW
                            ghs = ed.tile([128, T_e], F32, tag="ghs")
                            nc.gpsimd.ap_gather(
                                ghs[:], table[:], idxs_sb[:, o:o + T16],
                                channels=128, num_elems=NLP, d=1, num_idxs=T_e)
                            ade = e1.tile([128, T_e], F16, tag="adg", bufs=3)
                            nc.sync.dma_start(
                                out=ade[:], in_=adeg[l][:, t * T_e:(t + 1) * T_e])
                            msk = e1.tile([128, T_e], F16, tag="mask", bufs=3)
                            nc.sync.dma_start(
                                out=msk[:], in_=maskp[:, t * T_e:(t + 1) * T_e])
                            pend[t] = (ghs, ade, msk)

                        def process(t):
                            ghs, ade, msk = pend.pop(t)
                            o = t * IW
                            i_bnd = idxs_sb[:, o + T16:o + IW]
                            g16 = ghs[:].bitcast(F16)   # [128, 2*T_e]
                            tt = ed.tile([128, T_e], F16, tag="tt")
                            nc.vector.tensor_add(tt[:], g16[:, 1::2], ade[:])
                            nc.vector.scalar_tensor_tensor(
                                tt[:], tt[:], NEG_SLOPE, tt[:],
                                op0=mybir.AluOpType.mult, op1=mybir.AluOpType.max)
                            pp = e1.tile([128, T_e], F16, tag="pp", bufs=2)
                            nc.scalar.activation(pp[:], tt[:],
                                                 mybir.ActivationFunctionType.Exp)
                            ww = ed.tile([128, T_e], F16, tag="ww")
                            nc.vector.tensor_mul(ww[:], pp[:], g16[:, 0::2])
                            sc = eb.tile([128, T_e], F32, tag="sc", bufs=2)
                            sc16 = sc[:].bitcast(F16)   # (w | p) lanes
                            nc.vector.tensor_tensor_scan(
                                sc16[:, 0::2], msk[:], ww[:], 0.0,
                                mybir.AluOpType.mult, mybir.AluOpType.add)
                            nc.vector.tensor_tensor_scan(
                                sc16[:, 1::2], msk[:], pp[:], 0.0,
                                mybir.AluOpType.mult, mybir.AluOpType.add)
                            bg = e1.tile([128, 512], F32, tag="bg", bufs=2)
                            nc.gpsimd.ap_gather(
                                bg[:], sc[:], i_bnd,
                                channels=128, num_elems=T_e, d=1, num_idxs=512)
                            bg16 = bg[:].bitcast(F16)
                            pu = edp.tile([16, 512], F32, tag="pu")
                            nc.tensor.matmul(pu[:], lhsT=selt[:], rhs=bg16[:, 0::2],
                                             start=True, stop=True)
                            ps = edp.tile([16, 512], F32, tag="ps")
                            nc.tensor.matmul(ps[:], lhsT=selt[:], rhs=bg16[:, 1::2],
                                             start=True, stop=True)
                            # add analytic self-loop terms, then normalize
                            den = ed.tile([16, NB], F32, tag="s16", bufs=4)
                            nc.vector.tensor_add(den[:], ps[:, :NB],
                                                 pselfv[:, t * NB:(t + 1) * NB])
                            rc = ed.tile([16, NB], F32, tag="s16", bufs=4)
                            nc.vector.reciprocal_approx_fast(out=rc[:], in_=den[:])
                            num = ed.tile([16, NB], F32, tag="s16", bufs=4)
                            nc.vector.tensor_add(num[:], pu[:, :NB],
                                                 wselfv[:, t * NB:(t + 1) * NB])
                            uv = ed.tile([16, NB], F32, tag="s16", bufs=4)
                            nc.vector.tensor_mul(uv[:], num[:], rc[:])
                            writer(t, uv)

                        issue(0)
                        for t in range(1, NT):
                            issue(t)
                            process(t - 1)
                        process(NT - 1)

            def write1(t, uv):
                nc.scalar.activation(out1v[:, t * NB:(t + 1) * NB], uv[:],
                                     mybir.ActivationFunctionType.Relu,
                                     bias=b1t[:, 0:1])

            def write2(t, uv):
                nc.vector.tensor_scalar_add(out2v[:, t * NB:(t + 1) * NB], uv[:],
                                            b2t[:, 0:1])

            run_layer(0, write1)

            # ---------------- phase A (layer 2): tables from out1T --------
            with (
                tc.tile_pool(name="pa2", bufs=2) as pa2,
                tc.tile_pool(name="pap2", bufs=2, space="PSUM") as pap2,
            ):
                for c in range(NCH):
                    sl = slice(c * CH, (c + 1) * CH)
                    ph = pap2.tile([16, CH], F32, tag="ph2")
                    nc.tensor.matmul(ph[:], lhsT=w2t[32:48, :], rhs=out1v[:, sl],
                                     start=True, stop=True)
                    h2ch = pa2.tile([16, CH], BF16, tag="h2ch")
                    nc.scalar.copy(h2ch[:], ph[:])
                    pas = pap2.tile([16, CH], F32, tag="pas2")
                    nc.tensor.matmul(pas[:], lhsT=a2t[:], rhs=h2ch[:], start=True, stop=True)
                    pad_ = pap2.tile([16, CH], F32, tag="pad2")
                    nc.tensor.matmul(pad_[:], lhsT=ad2t[:], rhs=h2ch[:], start=True, stop=True)
                    packed = pa2.tile([16, CH], F32, tag="packed2")
                    pk = packed[:].bitcast(F16)
                    nc.vector.tensor_copy(pk[:, 0::2], h2ch[:])
                    nc.vector.tensor_copy(pk[:, 1::2], pas[:])
                    nc.gpsimd.dma_start(out=ag_in[1][:, sl], in_=packed[:])
                    adrow = pa2.tile([1, CH], F16, tag="adrow2")
                    nc.vector.tensor_copy(adrow[:], pad_[0:1, :])
                    nc.gpsimd.dma_start(out=ad_row[1][0:1, sl], in_=adrow[:])
                    adfull = pa2.tile([16, CH], F32, tag="adfull2")
                    nc.scalar.copy(adfull[:], pad_[:])
                    tself = pa2.tile([16, CH], F32, tag="tself2")
                    nc.vector.tensor_add(tself[:], pas[:], adfull[:])
                    nc.vector.scalar_tensor_tensor(
                        tself[:], tself[:], NEG_SLOPE, tself[:],
                        op0=mybir.AluOpType.mult, op1=mybir.AluOpType.max)
                    pselfc = pa2.tile([16, CH], F32, tag="pselfc2")
                    nc.scalar.activation(pselfc[:], tself[:],
                                         mybir.ActivationFunctionType.Exp)
                    nc.scalar.copy(pselfv[:, sl], pselfc[:])
                    nc.vector.tensor_mul(wselfv[:, sl], pselfc[:], h2ch[:])

            run_layer(1, write2)

            # ---------------- log_softmax + transpose + store -------------
            with (
                tc.tile_pool(name="fin", bufs=2) as fin,
                tc.tile_pool(name="finp", bufs=2, space="PSUM") as finp,
                tc.tile_pool(name="fino", bufs=1) as fino,
            ):
                nodemaj = fino.tile([128, NBLK, H], F32, tag="nodemaj")
                for r in range(0, NBLK, 16):
                    nb = min(16, NBLK - r)
                    ptp = finp.tile([128, 16, 16], F32, tag="ptp")
                    for j in range(nb):
                        nc.tensor.transpose(ptp[:, j, :],
                                            out2v[:, (r + j) * 128:(r + j + 1) * 128],
                                            idt[64:80, :])
                    nc.vector.tensor_copy(nodemaj[:, r:r + nb, :], ptp[:, 0:nb, :])
                mx = fin.tile([128, NBLK], F32, tag="mx")
                nc.vector.tensor_reduce(mx[:], nodemaj[:], axis=mybir.AxisListType.X,
                                        op=mybir.AluOpType.max)
                zz = fino.tile([128, NBLK, H], F32, tag="zz")
                nc.vector.tensor_sub(zz[:], nodemaj[:],
                                     mx[:, :, None].to_broadcast([128, NBLK, H]))
                es = fino.tile([128, NBLK, H], F32, tag="es")
                nc.scalar.activation(es[:], zz[:], mybir.ActivationFunctionType.Exp)
                sm = fin.tile([128, NBLK], F32, tag="sm")
                nc.vector.tensor_reduce(sm[:], es[:], axis=mybir.AxisListType.X,
                                        op=mybir.AluOpType.add)
                ls = fin.tile([128, NBLK], F32, tag="ls")
                nc.scalar.activation(ls[:], sm[:], mybir.ActivationFunctionType.Ln)
                outf = fino.tile([128, NBLK, H], F32, tag="outf")
                nc.vector.tensor_sub(outf[:], zz[:],
                                     ls[:, :, None].to_broadcast([128, NBLK, H]))
                nc.gpsimd.dma_start(out=outp[:].rearrange("p (b h) -> p b h", h=H),
                                    in_=outf[:])

    nc.compile()
    n_reload = sum(
        1
        for f in nc.m.functions
        for bb in f.blocks
        for ins in bb.instructions
        if type(ins).__name__ == "InstPseudoReloadLibraryIndex"
    )
    assert n_reload <= 8, f"library thrash: {n_reload} reloads"
    if split:
        split_waits(nc, max_waits=max_waits, ctrl_max_waits=ctrl_max_waits)
    return nc


CTRL_TYPES = ("InstDrain", "InstNoOp", "InstHalt", "InstEventSemaphore")


def split_waits(nc, max_waits=2, ctrl_max_waits=1):
    """walrus in this container caps sync-waits per instruction; move excess
    waits onto preceding same-engine NoOps (each carrying one wait)."""
    for f in nc.m.functions:
        for bb in f.blocks:
            new_insts, changed = [], False
            for ins in bb.instructions:
                si = ins.sync_info
                cap = (ctrl_max_waits if type(ins).__name__ in CTRL_TYPES
                       else max_waits)
                if si is not None and si.on_wait is not None and len(si.on_wait) > cap:
                    waits = list(si.on_wait)
                    excess, keep = waits[:-cap] if cap else waits, waits[-cap:] if cap else []
                    for i, w in enumerate(excess):
                        nop = mybir.InstNoOp(name=f"{ins.name}-ws{i}", ins=[], outs=[])
                        nop.engine = ins.engine
                        nop.sync_info = mybir.SyncInfo(on_wait=[w], on_update=[])
                        new_insts.append(nop)
                    si.on_wait = keep
                    changed = True
                new_insts.append(ins)
            if changed:
                bb.instructions = new_insts
    # verify the rewrite stuck (pyo3 lists can copy-on-read)
    for f in nc.m.functions:
        for bb in f.blocks:
            for ins in bb.instructions:
                si = ins.sync_info
                cap = (ctrl_max_waits if type(ins).__name__ in CTRL_TYPES
                       else max_waits)
                assert si is None or si.on_wait is None or len(si.on_wait) <= cap, \
                    f"{ins.name}: {len(si.on_wait)} waits > {cap}"


# ------------------------------------------------------------ input packing

def make_in_maps(inputs, cfg, per_core):
    NC, NL, H, F = cfg["NCORES"], cfg["NL"], cfg["H"], cfg["F"]
    NB, CH, NCH = cfg["NB"], cfg["CH"], cfg["NCH"]
    KB = F // 128
    x = np.asarray(inputs["x"], dtype=np.float32)
    sel = np.zeros((128, 16), dtype=np.float32)
    sel[np.arange(128), np.arange(128) % 16] = 1.0
    shared = {
        "w1": np.ascontiguousarray(np.asarray(inputs["W1"], np.float32)),
        "w2": np.ascontiguousarray(np.asarray(inputs["W2"], np.float32)),
        "a1rep": np.ascontiguousarray(np.repeat(np.asarray(inputs["a_src1"], np.float32)[:, None], 16, 1)),
        "ad1rep": np.ascontiguousarray(np.repeat(np.asarray(inputs["a_dst1"], np.float32)[:, None], 16, 1)),
        "a2rep": np.ascontiguousarray(np.repeat(np.asarray(inputs["a_src2"], np.float32)[:, None], 16, 1)),
        "ad2rep": np.ascontiguousarray(np.repeat(np.asarray(inputs["a_dst2"], np.float32)[:, None], 16, 1)),
        "b1p": np.ascontiguousarray(np.asarray(inputs["b1"], np.float32)[:, None]),
        "b2p": np.ascontiguousarray(np.asarray(inputs["b2"], np.float32)[:, None]),
    }
    shared.update({
        "selp": sel,
        "identp": np.eye(16, dtype=np.float32),
    })
    in_maps = []
    for c in range(NC):
        m = dict(shared)
        xpart = x[c * NL:(c + 1) * NL]                    # [NL, F]
        xtw = (xpart.T.reshape(KB, 128, NCH, CH)
               .transpose(1, 2, 0, 3).reshape(128, NCH * KB * CH))
        m["xtw"] = np.ascontiguousarray(xtw)
        m["idxs"] = per_core[c]["idxs"]
        m["maskp"] = per_core[c]["maskp"]
        m["seedp"] = per_core[c]["seedp"]
        in_maps.append(m)
    return in_maps


def unshard_output(results, cfg):
    NC, NL, H = cfg["NCORES"], cfg["NL"], cfg["H"]
    NBLK = math.ceil(NL / 128)
    parts = []
    for c in range(NC):
        a = np.asarray(results[c]["out"]).reshape(128, NBLK, H)
        a = a.transpose(1, 0, 2).reshape(NBLK * 128, H)[:NL]
        parts.append(a)
    return np.concatenate(parts, axis=0)


# ------------------------------------------------------------------- driver

_CACHE = {}


def run_on_hw(inputs, cfg, trace=False, tmpdir=None):
    import os
    import shutil
    from concourse.bass_utils import run_bass_kernel_spmd
    if tmpdir is not None and os.path.isdir(tmpdir):
        shutil.rmtree(tmpdir, ignore_errors=True)
    if tmpdir is not None:
        os.makedirs(tmpdir, exist_ok=True)
    T_e, per_core = host_prep(inputs["edge_index"], cfg)
    key = (cfg["N"], T_e)
    if key not in _CACHE:
        _CACHE[key] = build_nc(cfg, T_e)
    nc = _CACHE[key]
    in_maps = make_in_maps(inputs, cfg, per_core)
    res = run_bass_kernel_spmd(nc, in_maps, list(range(cfg["NCORES"])),
                               trace=trace, tmpdir=tmpdir)
    out = unshard_output(res.results, cfg)
    return out, res


def kernel(**inputs):
    out, _ = run_on_hw(inputs, FULL_CFG)
    return out.astype(np.float32)


# revision 7
# speedup vs baseline: 1.8443x; 1.0106x over previous
"""GAT (2-layer, heads=1) on 8 Trainium2 NeuronCores.

Strategy (1D node partition):
  - Nodes are split into 8 chunks of NL; core c owns dst-chunk c.
  - Layer tables h^T/alpha_src are computed per-chunk (feature-major via
    PE matmuls on x^T), AllGathered, and kept in SBUF as a per-partition-group
    table: partition group g (16 partitions) holds (h|alpha_src) of chunk g,
    packed two fp16 per fp32 word.
  - Edges are bucketed on the host by (dst-core, src-chunk, dst-block) and
    sorted by dst. Each edge tile covers one block of NB dst nodes for all
    8 src-chunk groups at once; per-edge h|alpha_src gathers run on GPSIMD
    (ap_gather), per-edge math on DVE/ACT, and segment-sums via masked
    tensor_tensor_scan (state = mask*state + w, mask=0 at segment starts)
    + a boundary gather at each node's last edge. Group partials are
    combined with a matmul against a 0/1 selection matrix; softmax
    normalization divides at the node level.
  - Per-edge alpha_dst values are NOT gathered. Instead, for each tile a
    GPSIMD local_scatter places each dst-slot's alpha_dst (fp16) at that
    slot's first-edge stream position (host-known structure; per-partition
    independent indices; ~3.4us per 512-idx scatter vs ~64us for the old
    per-edge ap_gather), and a masked scan broadcasts the value along the
    run. The run-start mask itself is host-precomputed structure, DMA'd
    per tile (HWDGE), freeing GPSIMD entirely.
  - local_scatter and ap_gather live in different GPSIMD IRAM libraries, so
    each layer runs all local_scatters in a "seed phase" (results staged in
    DRAM), then all ap_gathers in the edge phase, with a scheduler fence
    between them -> 2 library reloads per layer instead of per-tile thrash.
  - Slot 0 of every bucket is a sentinel; nodes with no edges in a bucket
    point their boundary index at slot 0 (whose running sum is always 0).
  - Softmax max-subtraction is skipped: attention logits here are O(1), and
    alpha = exp(e)/sum(exp(e)) is shift-invariant.

Host preprocessing only reorders/buckets edge indices and emits 0/1 masks
(structure), never touches float data dependent on device results.
"""

import math

import ml_dtypes
import numpy as np

from concourse import bass, bacc, mybir
import concourse.tile as tile

F32 = mybir.dt.float32
BF16 = mybir.dt.bfloat16
F16 = mybir.dt.float16
I16 = mybir.dt.int16
I32 = mybir.dt.int32

NEG_SLOPE = 0.2
# fp32 word 0xF7500000: low fp16 lane (h) = 0, high fp16 lane
# (alpha_src) = -29952 -> exp(lrelu(...)) == 0, so sentinel edges vanish.
SENT_PACKED = float(
    np.frombuffer(
        (np.uint32(np.float16(-29952.0).view(np.uint16)) << np.uint32(16)).tobytes(),
        np.float32,
    )[0]
)

FULL_CFG = dict(
    NCORES=8, N=100000, F=512, H=16,
    NL=12500, NB=500, NT=25, CH=500, NCH=25,
)

LS_MAX = 2046   # local_scatter num_elems limit (num_elems*32 < 2^16)


def _halves(T_e):
    nh = math.ceil(T_e / LS_MAX)
    w0 = (T_e + nh - 1) // nh
    w0 = (w0 + 1) // 2 * 2
    return nh, w0


# ---------------------------------------------------------------- host prep

def _round_up(x, m):
    return (x + m - 1) // m * m


def host_prep(edge_index, cfg):
    """Bucket and sort edges; build device index streams and masks
    (structure only, no float data).

    Returns (T_e, per_core) where per_core[c] has:
      'idxs'  [128, NT*(T16+32)] i16   (src-local gather idx + boundary idx)
      'maskp' [128, NT*T_e]      f16   (1 inside a run, 0 at run starts)
      'seedp' [128, NT*NH*512]   i16   (run-start positions per dst slot,
                                        split into NH stream halves; -1 = none)
    """
    NC, N, NL = cfg["NCORES"], cfg["N"], cfg["NL"]
    NB, NT = cfg["NB"], cfg["NT"]
    G = 8

    # Self-loops are NOT added here: their contribution is computed
    # analytically at the node level on-device (no gather needed).
    src = np.asarray(edge_index[0], dtype=np.int64)
    dst = np.asarray(edge_index[1], dtype=np.int64)

    core = dst // NL
    grp = src // NL
    order = np.lexsort((src, dst, grp, core))
    src, dst, core, grp = (a[order] for a in (src, dst, core, grp))
    blk = (dst % NL) // NB

    bucket = (core * G + grp) * NT + blk
    nbuck = NC * G * NT
    counts = np.bincount(bucket, minlength=nbuck)
    # +1: slot 0 of every bucket is a sentinel (known-zero running sum)
    T_e = _round_up(int(counts.max()) + 17, 128)
    assert T_e <= 32767
    NH, W0 = _halves(T_e)

    starts = np.zeros(nbuck, dtype=np.int64)
    starts[1:] = np.cumsum(counts)[:-1]
    pos = np.arange(src.size) - starts[bucket] + 1

    is_last = np.ones(src.size, dtype=bool)
    is_last[:-1] = ~((bucket[1:] == bucket[:-1]) & (dst[1:] == dst[:-1]))
    is_first = np.ones(src.size, dtype=bool)
    is_first[1:] = ~((bucket[1:] == bucket[:-1]) & (dst[1:] == dst[:-1]))

    srcl = np.full((NC, G, NT, T_e), NL, dtype=np.int16)
    bnd = np.zeros((NC, G, NT, 512), dtype=np.int16)  # default: sentinel slot 0
    mask = np.ones((NC, G, NT, T_e), dtype=np.float16)
    mask[..., 0] = 0.0
    seed = np.full((NC, G, NT, NH, 512), -1, dtype=np.int16)

    c_, g_, b_ = core, grp, blk
    dloc = (dst % NL) % NB
    srcl[c_, g_, b_, pos] = (src % NL).astype(np.int16)
    bnd[c_[is_last], g_[is_last], b_[is_last],
        dloc[is_last]] = pos[is_last].astype(np.int16)
    mask[c_[is_first], g_[is_first], b_[is_first], pos[is_first]] = 0.0
    p1 = pos[is_first]
    hh = (p1 // W0).astype(np.int64)
    seed[c_[is_first], g_[is_first], b_[is_first], hh,
         dloc[is_first]] = (p1 - hh * W0).astype(np.int16)

    def wrap(a, w):
        # [NC, G, NT, w] -> [NC, 128, NT, w//16]; w16[c, 16g+p, t, s] = a[c, g, t, s*16+p]
        n = a.shape[-1]
        return (a.reshape(NC, G, NT, n // 16, 16)
                 .transpose(0, 1, 4, 2, 3)
                 .reshape(NC, 128, NT, n // 16))

    src_w, bnd_w = wrap(srcl, T_e), wrap(bnd, 512)
    idxs = np.concatenate([src_w, bnd_w], axis=3)  # [NC,128,NT,IW]
    IW = idxs.shape[3]
    idxs = np.ascontiguousarray(idxs.reshape(NC, 128, NT * IW))

    # per-partition replication within each 16-partition group
    maskw = np.ascontiguousarray(
        np.repeat(mask, 16, axis=1).reshape(NC, 128, NT * T_e))
    seedw = np.ascontiguousarray(
        np.repeat(seed.reshape(NC, G, NT, NH * 512), 16, axis=1)
        .reshape(NC, 128, NT * NH * 512))

    per_core = [{"idxs": idxs[c], "maskp": maskw[c], "seedp": seedw[c]}
                for c in range(NC)]
    return T_e, per_core


# ------------------------------------------------------------- device build

def build_nc(cfg, T_e, max_waits=2, ctrl_max_waits=1, split=True):
    NC, N, F, H = cfg["NCORES"], cfg["N"], cfg["F"], cfg["H"]
    NL, NB, NT = cfg["NL"], cfg["NB"], cfg["NT"]
    CH, NCH = cfg["CH"], cfg["NCH"]
    KB = F // 128
    NLP = NL + 16               # table width incl. sentinel
    T16 = T_e // 16
    IW = T16 + 512 // 16
    NH, W0 = _halves(T_e)
    NBLK = math.ceil(NL / 128)
    NL2 = NBLK * 128
    W2 = NL2 // 2               # fp32 words backing the bf16 out1T row
    rg = [list(range(NC))]

    nc = bacc.Bacc("TRN2", target_bir_lowering=False)

    xtw = nc.declare_dram_parameter("xtw", [128, NCH * KB * CH], BF16, isOutput=False)
    w1 = nc.declare_dram_parameter("w1", [F, H], F32, isOutput=False)
    w2 = nc.declare_dram_parameter("w2", [H, H], F32, isOutput=False)
    a1rep = nc.declare_dram_parameter("a1rep", [H, 16], F32, isOutput=False)
    ad1rep = nc.declare_dram_parameter("ad1rep", [H, 16], F32, isOutput=False)
    a2rep = nc.declare_dram_parameter("a2rep", [H, 16], F32, isOutput=False)
    ad2rep = nc.declare_dram_parameter("ad2rep", [H, 16], F32, isOutput=False)
    b1p = nc.declare_dram_parameter("b1p", [H, 1], F32, isOutput=False)
    b2p = nc.declare_dram_parameter("b2p", [H, 1], F32, isOutput=False)
    selp = nc.declare_dram_parameter("selp", [128, 16], F32, isOutput=False)
    identp = nc.declare_dram_parameter("identp", [16, 16], F32, isOutput=False)
    idxsp = nc.declare_dram_parameter("idxs", [128, NT * IW], I16, isOutput=False)
    maskp = nc.declare_dram_parameter("maskp", [128, NT * T_e], F16, isOutput=False)
    seedp = nc.declare_dram_parameter("seedp", [128, NT * NH * 512], I16,
                                      isOutput=False)
    outp = nc.declare_dram_parameter("out", [128, NBLK * H], F32, isOutput=True)

    ag_in = [nc.dram_tensor(f"ag_in{l}", [16, NL], F32) for l in (1, 2)]
    ag_out = [nc.dram_tensor(f"ag_out{l}", [128, NL], F32, addr_space="Shared")
              for l in (1, 2)]
    # per-node fp16 alpha_dst rows (own chunk), incl. zero pad wide enough
    # for the last tile's 512-wide adbt broadcast read
    NLP2 = max(NLP, (NT - 1) * NB + 512)
    ad_row = [nc.dram_tensor(f"ad_row{l}", [1, NLP2], F16) for l in (1, 2)]
    # per-edge alpha_dst streams staged by the seed phase
    adeg = [nc.dram_tensor(f"adeg{l}", [128, NT * T_e], F16) for l in (1, 2)]

    with tile.TileContext(nc, num_cores=NC) as tc:
        with tc.tile_pool(name="const", bufs=1) as cpool:
            w1t = cpool.tile([128, KB, H], BF16)
            nc.gpsimd.dma_start(out=w1t[:], in_=w1[:].rearrange("(b p) h -> p b h", p=128))
            # w2 / identity copies aligned to the partitions where out1T /
            # out2T live (matmul needs lhsT and rhs on the same partitions).
            w2t = cpool.tile([48, H], BF16)
            nc.gpsimd.dma_start(out=w2t[32:48, :], in_=w2[:])
            a1t = cpool.tile([16, 16], BF16)
            nc.gpsimd.dma_start(out=a1t[:], in_=a1rep[:])
            ad1t = cpool.tile([16, 16], BF16)
            nc.gpsimd.dma_start(out=ad1t[:], in_=ad1rep[:])
            a2t = cpool.tile([16, 16], BF16)
            nc.gpsimd.dma_start(out=a2t[:], in_=a2rep[:])
            ad2t = cpool.tile([16, 16], BF16)
            nc.gpsimd.dma_start(out=ad2t[:], in_=ad2rep[:])
            b1t = cpool.tile([16, 1], F32)
            nc.sync.dma_start(out=b1t[:], in_=b1p[:])
            b2t = cpool.tile([16, 1], F32)
            nc.sync.dma_start(out=b2t[:], in_=b2p[:])
            selt = cpool.tile([128, 16], F16)
            nc.gpsimd.dma_start(out=selt[:], in_=selp[:])
            idt = cpool.tile([80, 16], F32)
            nc.sync.dma_start(out=idt[64:80, :], in_=identp[:])

            # Stacked per-layer state, one tile so 16-partition rows share
            # column space; engine accesses must start at partition 0/32/64/96:
            # p0-15 wself, p32-47 out1T (bf16, matmul rhs -> base 32),
            # p64-79 out2T (transpose input -> base 64), p96-111 pself.
            stk = cpool.tile([128, NL2], F32)
            wselfv = stk[0:16, 0:NL]
            pselfv = stk[96:112, 0:NL]
            out1v = stk[32:48, 0:W2].bitcast(BF16)      # [16, NL2] bf16
            out2v = stk[64:80, 0:NL2]
            nc.vector.memset(stk[64:80, NL:], 0.0)

            zero16 = cpool.tile([1, NLP2 - NL], F16)
            nc.vector.memset(zero16[:], 0.0)

            idxs_sb = cpool.tile([128, NT * IW], I16)
            nc.gpsimd.dma_start(out=idxs_sb[:], in_=idxsp[:])

            # ---------------- phase A (layer 1): tables from x^T ----------
            with (
                tc.tile_pool(name="pa", bufs=2) as pa,
                tc.tile_pool(name="pap", bufs=2, space="PSUM") as pap,
            ):
                for c in range(NCH):
                    sl = slice(c * CH, (c + 1) * CH)
                    xt_t = pa.tile([128, KB, CH], BF16, tag="xt")
                    nc.sync.dma_start(
                        out=xt_t[:],
                        in_=xtw[:, c * KB * CH:(c + 1) * KB * CH]
                        .rearrange("p (b n) -> p b n", b=KB))
                    ph = pap.tile([16, CH], F32, tag="ph")
                    for b in range(KB):
                        nc.tensor.matmul(ph[:], lhsT=w1t[:, b, :], rhs=xt_t[:, b, :],
                                         start=(b == 0), stop=(b == KB - 1))
                    hch = pa.tile([16, CH], BF16, tag="hch")
                    nc.scalar.copy(hch[:], ph[:])
                    pas = pap.tile([16, CH], F32, tag="pas")
                    nc.tensor.matmul(pas[:], lhsT=a1t[:], rhs=hch[:], start=True, stop=True)
                    pad_ = pap.tile([16, CH], F32, tag="pad")
                    nc.tensor.matmul(pad_[:], lhsT=ad1t[:], rhs=hch[:], start=True, stop=True)
                    packed = pa.tile([16, CH], F32, tag="packed")
                    pk = packed[:].bitcast(F16)
                    nc.vector.tensor_copy(pk[:, 0::2], hch[:])
                    nc.vector.tensor_copy(pk[:, 1::2], pas[:])
                    nc.sync.dma_start(out=ag_in[0][:, sl], in_=packed[:])
                    adrow = pa.tile([1, CH], F16, tag="adrow")
                    nc.vector.tensor_copy(adrow[:], pad_[0:1, :])
                    nc.sync.dma_start(out=ad_row[0][0:1, sl], in_=adrow[:])
                    # analytic self-loop contribution for this chunk
                    adfull = pa.tile([16, CH], F32, tag="adfull")
                    nc.scalar.copy(adfull[:], pad_[:])
                    tself = pa.tile([16, CH], F32, tag="tself")
                    nc.vector.tensor_add(tself[:], pas[:], adfull[:])
                    nc.vector.scalar_tensor_tensor(
                        tself[:], tself[:], NEG_SLOPE, tself[:],
                        op0=mybir.AluOpType.mult, op1=mybir.AluOpType.max)
                    pselfc = pa.tile([16, CH], F32, tag="pselfc")
                    nc.scalar.activation(pselfc[:], tself[:],
                                         mybir.ActivationFunctionType.Exp)
                    nc.scalar.copy(pselfv[:, sl], pselfc[:])
                    nc.vector.tensor_mul(wselfv[:, sl], pselfc[:], hch[:])

            def run_layer(l, writer):
                nc.sync.dma_start(out=ad_row[l][0:1, NL:], in_=zero16[:])
                nc.gpsimd.collective_compute(
                    "AllGather", mybir.AluOpType.bypass, replica_groups=rg,
                    ins=[ag_in[l][:]], outs=[ag_out[l][:]])
                # ---- seed phase: alpha_dst -> run-start seeds -> scan ----
                with tc.tile_pool(name=f"sd{l}", bufs=3) as sd:
                    for t in range(NT):
                        adbt = sd.tile([128, 512], F16, tag="adbt")
                        nc.sync.dma_start(
                            out=adbt[:],
                            in_=ad_row[l][0:1, t * NB:t * NB + 512]
                            .to_broadcast([128, 512]))
                        sidx = sd.tile([128, NH * 512], I16, tag="sidx")
                        nc.sync.dma_start(
                            out=sidx[:],
                            in_=seedp[:, t * NH * 512:(t + 1) * NH * 512])
                        seeds = sd.tile([128, T_e], F16, tag="seeds")
                        for h in range(NH):
                            w0, w1_ = h * W0, min((h + 1) * W0, T_e)
                            nc.gpsimd.local_scatter(
                                seeds[:, w0:w1_], adbt[:],
                                sidx[:, h * 512:(h + 1) * 512],
                                channels=128, num_elems=w1_ - w0, num_idxs=512)
                        msk = sd.tile([128, T_e], F16, tag="msk")
                        nc.sync.dma_start(
                            out=msk[:], in_=maskp[:, t * T_e:(t + 1) * T_e])
                        ade = sd.tile([128, T_e], F16, tag="ade")
                        nc.vector.tensor_tensor_scan(
                            ade[:], msk[:], seeds[:], 0.0,
                            mybir.AluOpType.mult, mybir.AluOpType.add)
                        nc.sync.dma_start(
                            out=adeg[l][:, t * T_e:(t + 1) * T_e], in_=ade[:])
                tc.no_sync_barrier()
                # ---- edge phase: h|as gather + per-edge math + seg-sums ----
                with tc.tile_pool(name=f"tab{l}", bufs=1) as tabp:
                    table = tabp.tile([128, NLP], F32, tag="table")
                    nc.sync.dma_start(out=table[:, :NL], in_=ag_out[l][:])
                    nc.vector.memset(table[:, NL:], SENT_PACKED)
                    with (
                        tc.tile_pool(name=f"ed{l}", bufs=2) as ed,
                        tc.tile_pool(name=f"eb{l}", bufs=3) as eb,
                        tc.tile_pool(name=f"e1{l}", bufs=1) as e1,
                        tc.tile_pool(name=f"edp{l}", bufs=2, space="PSUM") as edp,
                    ):
                        # software-pipelined: issue tile t's h|as gather one
                        # iteration ahead of its per-edge math, so the DVE
                        # chain of tile t hides under the gather of t+1 and
                        # GPSIMD stays busy back-to-back.
                        pend = {}

                        def issue(t):
                            o = t * IW
                            ghs = ed.tile([128, T_e], F32, tag="ghs")
                            nc.gpsimd.ap_gather(
                                ghs[:], table[:], idxs_sb[:, o:o + T16],
                                channels=128, num_elems=NLP, d=1, num_idxs=T_e)
                            ade = e1.tile([128, T_e], F16, tag="adg", bufs=3)
                            nc.sync.dma_start(
                                out=ade[:], in_=adeg[l][:, t * T_e:(t + 1) * T_e])
                            msk = e1.tile([128, T_e], F16, tag="mask", bufs=3)
                            nc.sync.dma_start(
                                out=msk[:], in_=maskp[:, t * T_e:(t + 1) * T_e])
                            pend[t] = (ghs, ade, msk)

                        def process(t):
                            ghs, ade, msk = pend.pop(t)
                            o = t * IW
                            i_bnd = idxs_sb[:, o + T16:o + IW]
                            g16 = ghs[:].bitcast(F16)   # [128, 2*T_e]
                            tt = ed.tile([128, T_e], F16, tag="tt")
                            nc.vector.tensor_add(tt[:], g16[:, 1::2], ade[:])
                            nc.vector.scalar_tensor_tensor(
                                tt[:], tt[:], NEG_SLOPE, tt[:],
                                op0=mybir.AluOpType.mult, op1=mybir.AluOpType.max)
                            pp = e1.tile([128, T_e], F16, tag="pp", bufs=2)
                            nc.scalar.activation(pp[:], tt[:],
                                                 mybir.ActivationFunctionType.Exp)
                            ww = ed.tile([128, T_e], F16, tag="ww")
                            nc.vector.tensor_mul(ww[:], pp[:], g16[:, 0::2])
                            sc = eb.tile([128, T_e], F32, tag="sc", bufs=2)
                            sc16 = sc[:].bitcast(F16)   # (w | p) lanes
                            nc.vector.tensor_tensor_scan(
                                sc16[:, 0::2], msk[:], ww[:], 0.0,
                                mybir.AluOpType.mult, mybir.AluOpType.add)
                            nc.vector.tensor_tensor_scan(
                                sc16[:, 1::2], msk[:], pp[:], 0.0,
                                mybir.AluOpType.mult, mybir.AluOpType.add)
                            bg = e1.tile([128, 512], F32, tag="bg", bufs=2)
                            nc.gpsimd.ap_gather(
                                bg[:], sc[:], i_bnd,
                                channels=128, num_elems=T_e, d=1, num_idxs=512)
                            bg16 = bg[:].bitcast(F16)
                            pu = edp.tile([16, 512], F32, tag="pu")
                            nc.tensor.matmul(pu[:], lhsT=selt[:], rhs=bg16[:, 0::2],
                                             start=True, stop=True)
                            ps = edp.tile([16, 512], F32, tag="ps")
                            nc.tensor.matmul(ps[:], lhsT=selt[:], rhs=bg16[:, 1::2],
                                             start=True, stop=True)
                            # add analytic self-loop terms, then normalize
                            den = ed.tile([16, NB], F32, tag="s16", bufs=4)
                            nc.vector.tensor_add(den[:], ps[:, :NB],
                                                 pselfv[:, t * NB:(t + 1) * NB])
                            rc = ed.tile([16, NB], F32, tag="s16", bufs=4)
                            nc.vector.reciprocal_approx_fast(out=rc[:], in_=den[:])
                            num = ed.tile([16, NB], F32, tag="s16", bufs=4)
                            nc.vector.tensor_add(num[:], pu[:, :NB],
                                                 wselfv[:, t * NB:(t + 1) * NB])
                            uv = ed.tile([16, NB], F32, tag="s16", bufs=4)
                            nc.vector.tensor_mul(uv[:], num[:], rc[:])
                            writer(t, uv)

                        issue(0)
                        for t in range(1, NT):
                            issue(t)
                            process(t - 1)
                        process(NT - 1)

            def write1(t, uv):
                nc.scalar.activation(out1v[:, t * NB:(t + 1) * NB], uv[:],
                                     mybir.ActivationFunctionType.Relu,
                                     bias=b1t[:, 0:1])

            def write2(t, uv):
                nc.vector.tensor_scalar_add(out2v[:, t * NB:(t + 1) * NB], uv[:],
                                            b2t[:, 0:1])

            run_layer(0, write1)

            # ---------------- phase A (layer 2): tables from out1T --------
            with (
                tc.tile_pool(name="pa2", bufs=2) as pa2,
                tc.tile_pool(name="pap2", bufs=2, space="PSUM") as pap2,
            ):
                for c in range(NCH):
                    sl = slice(c * CH, (c + 1) * CH)
                    ph = pap2.tile([16, CH], F32, tag="ph2")
                    nc.tensor.matmul(ph[:], lhsT=w2t[32:48, :], rhs=out1v[:, sl],
                                     start=True, stop=True)
                    h2ch = pa2.tile([16, CH], BF16, tag="h2ch")
                    nc.scalar.copy(h2ch[:], ph[:])
                    pas = pap2.tile([16, CH], F32, tag="pas2")
                    nc.tensor.matmul(pas[:], lhsT=a2t[:], rhs=h2ch[:], start=True, stop=True)
                    pad_ = pap2.tile([16, CH], F32, tag="pad2")
                    nc.tensor.matmul(pad_[:], lhsT=ad2t[:], rhs=h2ch[:], start=True, stop=True)
                    packed = pa2.tile([16, CH], F32, tag="packed2")
                    pk = packed[:].bitcast(F16)
                    nc.vector.tensor_copy(pk[:, 0::2], h2ch[:])
                    nc.vector.tensor_copy(pk[:, 1::2], pas[:])
                    nc.sync.dma_start(out=ag_in[1][:, sl], in_=packed[:])
                    adrow = pa2.tile([1, CH], F16, tag="adrow2")
                    nc.vector.tensor_copy(adrow[:], pad_[0:1, :])
                    nc.sync.dma_start(out=ad_row[1][0:1, sl], in_=adrow[:])
                    adfull = pa2.tile([16, CH], F32, tag="adfull2")
                    nc.scalar.copy(adfull[:], pad_[:])
                    tself = pa2.tile([16, CH], F32, tag="tself2")
                    nc.vector.tensor_add(tself[:], pas[:], adfull[:])
                    nc.vector.scalar_tensor_tensor(
                        tself[:], tself[:], NEG_SLOPE, tself[:],
                        op0=mybir.AluOpType.mult, op1=mybir.AluOpType.max)
                    pselfc = pa2.tile([16, CH], F32, tag="pselfc2")
                    nc.scalar.activation(pselfc[:], tself[:],
                                         mybir.ActivationFunctionType.Exp)
                    nc.scalar.copy(pselfv[:, sl], pselfc[:])
                    nc.vector.tensor_mul(wselfv[:, sl], pselfc[:], h2ch[:])

            run_layer(1, write2)

            # ---------------- log_softmax + transpose + store -------------
            with (
                tc.tile_pool(name="fin", bufs=2) as fin,
                tc.tile_pool(name="finp", bufs=2, space="PSUM") as finp,
                tc.tile_pool(name="fino", bufs=1) as fino,
            ):
                nodemaj = fino.tile([128, NBLK, H], F32, tag="nodemaj")
                for r in range(0, NBLK, 16):
                    nb = min(16, NBLK - r)
                    ptp = finp.tile([128, 16, 16], F32, tag="ptp")
                    for j in range(nb):
                        nc.tensor.transpose(ptp[:, j, :],
                                            out2v[:, (r + j) * 128:(r + j + 1) * 128],
                                            idt[64:80, :])
                    nc.vector.tensor_copy(nodemaj[:, r:r + nb, :], ptp[:, 0:nb, :])
                mx = fin.tile([128, NBLK], F32, tag="mx")
                nc.vector.tensor_reduce(mx[:], nodemaj[:], axis=mybir.AxisListType.X,
                                        op=mybir.AluOpType.max)
                zz = fino.tile([128, NBLK, H], F32, tag="zz")
                nc.vector.tensor_sub(zz[:], nodemaj[:],
                                     mx[:, :, None].to_broadcast([128, NBLK, H]))
                es = fino.tile([128, NBLK, H], F32, tag="es")
                nc.scalar.activation(es[:], zz[:], mybir.ActivationFunctionType.Exp)
                sm = fin.tile([128, NBLK], F32, tag="sm")
                nc.vector.tensor_reduce(sm[:], es[:], axis=mybir.AxisListType.X,
                                        op=mybir.AluOpType.add)
                ls = fin.tile([128, NBLK], F32, tag="ls")
                nc.scalar.activation(ls[:], sm[:], mybir.ActivationFunctionType.Ln)
                outf = fino.tile([128, NBLK, H], F32, tag="outf")
                nc.vector.tensor_sub(outf[:], zz[:],
                                     ls[:, :, None].to_broadcast([128, NBLK, H]))
                nc.gpsimd.dma_start(out=outp[:].rearrange("p (b h) -> p b h", h=H),
                                    in_=outf[:])

    nc.compile()
    n_reload = sum(
        1
        for f in nc.m.functions
        for bb in f.blocks
        for ins in bb.instructions
        if type(ins).__name__ == "InstPseudoReloadLibraryIndex"
    )
    assert n_reload <= 8, f"library thrash: {n_reload} reloads"
    if split:
        split_waits(nc, max_waits=max_waits, ctrl_max_waits=ctrl_max_waits)
    return nc


CTRL_TYPES = ("InstDrain", "InstNoOp", "InstHalt", "InstEventSemaphore")


def split_waits(nc, max_waits=2, ctrl_max_waits=1):
    """walrus in this container caps sync-waits per instruction; move excess
    waits onto preceding same-engine NoOps (each carrying one wait)."""
    for f in nc.m.functions:
        for bb in f.blocks:
            new_insts, changed = [], False
            for ins in bb.instructions:
                si = ins.sync_info
                cap = (ctrl_max_waits if type(ins).__name__ in CTRL_TYPES
                       else max_waits)
                if si is not None and si.on_wait is not None and len(si.on_wait) > cap:
                    waits = list(si.on_wait)
                    excess, keep = waits[:-cap] if cap else waits, waits[-cap:] if cap else []
                    for i, w in enumerate(excess):
                        nop = mybir.InstNoOp(name=f"{ins.name}-ws{i}", ins=[], outs=[])
                        nop.engine = ins.engine
                        nop.sync_info = mybir.SyncInfo(on_wait=[w], on_update=[])
                        new_insts.append(nop)
                    si.on_wait = keep
                    changed = True
                new_insts.append(ins)
            if changed:
                bb.instructions = new_insts
    # verify the rewrite stuck (pyo3 lists can copy-on-read)
    for f in nc.m.functions:
        for bb in f.blocks:
            for ins in bb.instructions:
                si = ins.sync_info
                cap = (ctrl_max_waits if type(ins).__name__ in CTRL_TYPES
                       else max_waits)
                assert si is None or si.on_wait is None or len(si.on_wait) <= cap, \
                    f"{ins.name}: {len(si.on_wait)} waits > {cap}"


# ------------------------------------------------------------ input packing

def make_in_maps(inputs, cfg, per_core):
    NC, NL, H, F = cfg["NCORES"], cfg["NL"], cfg["H"], cfg["F"]
    NB, CH, NCH = cfg["NB"], cfg["CH"], cfg["NCH"]
    KB = F // 128
    x = np.asarray(inputs["x"], dtype=np.float32)
    sel = np.zeros((128, 16), dtype=np.float32)
    sel[np.arange(128), np.arange(128) % 16] = 1.0
    shared = {
        "w1": np.ascontiguousarray(np.asarray(inputs["W1"], np.float32)),
        "w2": np.ascontiguousarray(np.asarray(inputs["W2"], np.float32)),
        "a1rep": np.ascontiguousarray(np.repeat(np.asarray(inputs["a_src1"], np.float32)[:, None], 16, 1)),
        "ad1rep": np.ascontiguousarray(np.repeat(np.asarray(inputs["a_dst1"], np.float32)[:, None], 16, 1)),
        "a2rep": np.ascontiguousarray(np.repeat(np.asarray(inputs["a_src2"], np.float32)[:, None], 16, 1)),
        "ad2rep": np.ascontiguousarray(np.repeat(np.asarray(inputs["a_dst2"], np.float32)[:, None], 16, 1)),
        "b1p": np.ascontiguousarray(np.asarray(inputs["b1"], np.float32)[:, None]),
        "b2p": np.ascontiguousarray(np.asarray(inputs["b2"], np.float32)[:, None]),
    }
    shared.update({
        "selp": sel,
        "identp": np.eye(16, dtype=np.float32),
    })
    in_maps = []
    for c in range(NC):
        m = dict(shared)
        xpart = x[c * NL:(c + 1) * NL]                    # [NL, F]
        xtw = (xpart.T.reshape(KB, 128, NCH, CH)
               .transpose(1, 2, 0, 3).reshape(128, NCH * KB * CH))
        m["xtw"] = np.ascontiguousarray(xtw.astype(ml_dtypes.bfloat16))
        m["idxs"] = per_core[c]["idxs"]
        m["maskp"] = per_core[c]["maskp"]
        m["seedp"] = per_core[c]["seedp"]
        in_maps.append(m)
    return in_maps


def unshard_output(results, cfg):
    NC, NL, H = cfg["NCORES"], cfg["NL"], cfg["H"]
    NBLK = math.ceil(NL / 128)
    parts = []
    for c in range(NC):
        a = np.asarray(results[c]["out"]).reshape(128, NBLK, H)
        a = a.transpose(1, 0, 2).reshape(NBLK * 128, H)[:NL]
        parts.append(a)
    return np.concatenate(parts, axis=0)


# ------------------------------------------------------------------- driver

_CACHE = {}


def run_on_hw(inputs, cfg, trace=False, tmpdir=None):
    import os
    import shutil
    from concourse.bass_utils import run_bass_kernel_spmd
    if tmpdir is not None and os.path.isdir(tmpdir):
        shutil.rmtree(tmpdir, ignore_errors=True)
    if tmpdir is not None:
        os.makedirs(tmpdir, exist_ok=True)
    T_e, per_core = host_prep(inputs["edge_index"], cfg)
    key = (cfg["N"], T_e)
    if key not in _CACHE:
        _CACHE[key] = build_nc(cfg, T_e)
    nc = _CACHE[key]
    in_maps = make_in_maps(inputs, cfg, per_core)
    res = run_bass_kernel_spmd(nc, in_maps, list(range(cfg["NCORES"])),
                               trace=trace, tmpdir=tmpdir)
    out = unshard_output(res.results, cfg)
    return out, res


def kernel(**inputs):
    out, _ = run_on_hw(inputs, FULL_CFG)
    return out.astype(np.float32)
